# revision 13
# baseline (speedup 1.0000x reference)
"""CosineSimilarityAttention Trainium2 kernel (8 NeuronCores, SPMD).

Sharding: token-parallel. Global tokens = 2 batches x 4096. Core c handles
batch (c // 4), query rows (c % 4)*1024 .. +1024. Each core computes K/V
projections for its whole batch (4096 tokens), plus Q for its own 1024
tokens, then attention and the output projection for its token slice.

Math per batch (faithful to reference):
  qkv = x @ w_qkv.T ; split q,k,v ; reshape heads h=12, dh=64
  q *= 1/sqrt(||q||_heads + eps)   (L2 norm over the HEADS axis, per (n, dh))
  k *= 1/sqrt(||k||_heads + eps)
  out_h = softmax((q_h k_h^T) / scale_h) v_h   (no max-subtract: |logits|<~2)
  y = concat_h(out_h) @ w_out.T + b_out

Perf structure (v2):
  - scores matmuls are K=64 per head; head pairs run as concurrent PE
    row-tiles (rows 0-63 / 64-127) instead of zero-padded K=128 matmuls.
  - head-norm rsqrt chain is Sqrt + Rsqrt(bias=eps) on ScalarE (no DVE
    reciprocal); squares come from PSUM via ScalarE Square.
  - softmax denominators accumulate via the ones-column of vhat (65-wide
    attnV tiles); normalization is batched: one DVE reciprocal over all
    24 (head, q-half) rows, then 12 selector matmuls broadcast 1/den.
  - output projection contracts head PAIRS (K=128) directly from the
    pair-stacked attention-output layout.
  - V projection is folded into the per-block W+P loop.
"""

import numpy as np

import concourse.bass as bass
import concourse.mybir as mybir
import concourse.tile as tile
from concourse.bass_utils import run_bass_kernel_spmd
from concourse.masks import make_identity

F32 = mybir.dt.float32
BF16 = mybir.dt.bfloat16
AF = mybir.ActivationFunctionType

B = 2
N = 4096          # tokens per batch
D = 768           # model dim
H = 12            # heads
DH = 64           # head dim
INNER = H * DH    # 768
EPS = 1e-8
NQ = 1024         # query tokens per core
NCORES = 8
BLK = 512         # projection token block
KT = N // 128     # 32 key tiles of 128


def _split_multi_waits(nc):
    """This container's walrus accepts only ONE sync-wait per instruction.
    Hoist extra waits into standalone EVSEM instructions placed just before."""
    n = 0
    for f in nc.m.functions:
        for bb in f.blocks:
            insts = list(bb.instructions)
            out = []
            for inst in insts:
                si = inst.sync_info
                if si is not None and si.on_wait is not None and len(si.on_wait) > 1:
                    waits = list(si.on_wait)
                    for j, w in enumerate(waits[:-1]):
                        ev = mybir.InstEventSemaphore(
                            name=f"{inst.name}-evw{j}",
                            engine=inst.engine,
                            sync_info=mybir.SyncInfo(on_wait=[w], on_update=[]),
                        )
                        out.append(ev)
                        n += 1
                    si.on_wait = [waits[-1]]
                out.append(inst)
            bb.instructions = out
    return n


def _build_program(inv_scale):
    """Build the single SPMD Bass program. inv_scale: list of 12 floats."""
    nc = bass.Bass()
    xb = nc.declare_dram_parameter("xb", [N, D], F32, isOutput=False)
    qx = nc.declare_dram_parameter("qx", [NQ, D], F32, isOutput=False)
    wqkvT = nc.declare_dram_parameter("wqkvT", [D, 3 * INNER], F32, isOutput=False)
    woT = nc.declare_dram_parameter("woT", [INNER, D], F32, isOutput=False)
    bout = nc.declare_dram_parameter("bout", [1, D], F32, isOutput=False)
    selin = nc.declare_dram_parameter("selin", [128, 128], F32, isOutput=False)
    selqin = nc.declare_dram_parameter("selqin", [128, 2 * 128], F32,
                                       isOutput=False)
    y = nc.declare_dram_parameter("y", [NQ, D], F32, isOutput=True)

    with tile.TileContext(nc) as tc:
        with tc.tile_pool(name="const", bufs=1) as constp, \
             tc.tile_pool(name="persist", bufs=1) as persist:
            # --- constants ---
            ident = constp.tile([128, 128], F32)
            make_identity(nc, ident)
            sel_st = constp.tile([128, 128], F32)
            nc.sync.dma_start(out=sel_st, in_=selin[:, :])
            sel_bf = constp.tile([128, 128], BF16)
            nc.vector.tensor_copy(sel_bf, sel_st)
            selq_bf = constp.tile([128, 2, 128], BF16)
            ones128 = constp.tile([128, 1], BF16)
            nc.vector.memset(ones128, 1.0)
            ones_bf = constp.tile([1, 128], BF16)
            nc.vector.memset(ones_bf, 1.0)
            eps_t = constp.tile([128, 1], F32)
            nc.vector.memset(eps_t, EPS)
            invs = constp.tile([128, 6], F32)
            for dt in range(6):
                nc.vector.memset(invs[0:64, dt:dt + 1], float(inv_scale[2 * dt]))
                nc.vector.memset(invs[64:128, dt:dt + 1],
                                 float(inv_scale[2 * dt + 1]))
            b_bf = constp.tile([1, D], BF16)
            with tc.tile_pool(name="cstage", bufs=1) as cst:
                selq_st = cst.tile([128, 2 * 128], F32)
                nc.sync.dma_start(out=selq_st, in_=selqin[:, :])
                nc.vector.tensor_copy(
                    selq_bf, selq_st.rearrange("p (i c) -> p i c", c=128))
                b_st = cst.tile([1, D], F32)
                nc.sync.dma_start(out=b_st, in_=bout[:, :])
                nc.vector.tensor_copy(b_bf, b_st)

            # --- persistent activations ---
            khat = persist.tile([128, 6, N], BF16)      # k^T normed [dim, tok]
            qhat = persist.tile([128, 6, NQ], BF16)     # q^T normed, pair-packed
            vhat = persist.tile([128, KT, H * DH], BF16)  # v [tok, h*dh]
            o2_all = persist.tile([128, 6, NQ], BF16)   # attn out, pair-stacked
            rinv_all = persist.tile([128, 6, 512], BF16)  # 1/den, rows 32*(2qh+j)
            wo12 = persist.tile([128, 6, D], BF16)      # w_out pair-stacked

            # output-projection weights: head h rows -> partitions (h%2)*64
            with tc.tile_pool(name="wostage", bufs=2) as wost:
                for h in range(H):
                    wst_t = wost.tile([64, D], F32, tag="wost")
                    nc.sync.dma_start(out=wst_t, in_=woT[h * 64:(h + 1) * 64, :])
                    base = (h % 2) * 64
                    nc.vector.tensor_copy(wo12[base:base + 64, h // 2, :], wst_t)

            # ---------------- phase W+P: weights, projections, head-norm ----
            with tc.tile_pool(name="pw", bufs=1) as pwp:
              wq = pwp.tile([128, 6, 3 * INNER], BF16)
              with tc.tile_pool(name="wstage", bufs=1) as wst:
                for dt in range(6):
                    st = wst.tile([128, 3 * INNER], F32, tag="wst")
                    nc.sync.dma_start(out=st, in_=wqkvT[dt * 128:(dt + 1) * 128, :])
                    nc.vector.tensor_copy(wq[:, dt, :], st)
              with tc.tile_pool(name="pstage", bufs=2) as pstage, \
                   tc.tile_pool(name="pxT", bufs=2) as pxT, \
                   tc.tile_pool(name="pkf", bufs=2) as pkf, \
                   tc.tile_pool(name="pksq", bufs=2) as pksq, \
                   tc.tile_pool(name="psmall", bufs=1) as psmall, \
                   tc.tile_pool(name="psumTP", bufs=1, space="PSUM") as pTP, \
                   tc.tile_pool(name="psumKP", bufs=2, space="PSUM") as pKP, \
                   tc.tile_pool(name="psumSQ", bufs=1, space="PSUM") as pSQ, \
                   tc.tile_pool(name="psumVP", bufs=2, space="PSUM") as pVP:

                  def proj_block(src, blk_i, is_q):
                      # load + transpose x block [512, D] -> xT [dim, tok] bf16
                      xsts = []
                      for hh in range(2):
                          xst = pstage.tile([128, 2, D], F32, tag="xst")
                          nc.sync.dma_start(
                              out=xst,
                              in_=src[blk_i * BLK + hh * 256:
                                      blk_i * BLK + (hh + 1) * 256, :].rearrange(
                                  "(t p) d -> p t d", p=128),
                          )
                          xsts.append(xst)
                      xT = pxT.tile([128, 6, BLK], BF16, tag="xT")
                      for dt in range(6):
                          tp = pTP.tile([128, 512], F32, tag="pTP")
                          for tt in range(4):
                              nc.tensor.transpose(
                                  tp[:, tt * 128:(tt + 1) * 128],
                                  xsts[tt // 2][:, tt % 2,
                                                dt * 128:(dt + 1) * 128], ident)
                          nc.vector.tensor_copy(xT[:, dt, :], tp)

                      wbase = 0 if is_q else INNER
                      # q^T / k^T projection [dim_out, tok]; kf bf16 via ScalarE,
                      # squares straight from PSUM via ScalarE Square
                      kf = pkf.tile([128, 6, BLK], BF16, tag="kf")
                      sq = pSQ.tile([128, 512], F32, tag="pSQ")
                      for dt in range(6):
                          kp = pKP.tile([128, 512], F32, tag="pKP")
                          for ks in range(6):
                              nc.tensor.matmul(
                                  kp,
                                  wq[:, ks, wbase + dt * 128: wbase + (dt + 1) * 128],
                                  xT[:, ks, :],
                                  start=(ks == 0), stop=(ks == 5))
                          nc.scalar.copy(kf[:, dt, :], kp)
                          ksq = pksq.tile([128, BLK], BF16, tag="ksq")
                          nc.scalar.square(ksq, kp)
                          nc.tensor.matmul(sq, sel_bf, ksq,
                                           start=(dt == 0), stop=(dt == 5))
                      nrm = psmall.tile([128, BLK], F32, tag="nrm")
                      nc.scalar.activation(nrm, sq, AF.Sqrt)
                      u = psmall.tile([128, BLK], F32, tag="u")
                      nc.scalar.activation(u, nrm, AF.Sqrt, bias=eps_t[:, :])
                      rq = psmall.tile([128, BLK], F32, tag="nrm")
                      nc.vector.reciprocal(rq, u)
                      bsl = bass.ts(blk_i, BLK)
                      if is_q:
                          # 1/scale_h folded into qhat so exp needs no scale
                          for dt in range(6):
                              nc.vector.tensor_mul(
                                  qhat[:, dt, bsl], kf[:, dt, :], rq)
                              nc.vector.tensor_scalar_mul(
                                  qhat[:, dt, bsl], qhat[:, dt, bsl],
                                  invs[:, dt:dt + 1])
                          return
                      for dt in range(6):
                          nc.vector.tensor_mul(
                              khat[:, dt, bsl], kf[:, dt, :], rq)
                      # v projection [tok, inner] -> vhat strided 65
                      for tt in range(4):
                          vp = pVP.tile([128, 1024], F32, tag="pVP")
                          for ks in range(6):
                              nc.tensor.matmul(vp[:, 0:512],
                                               xT[:, ks, tt * 128:(tt + 1) * 128],
                                               wq[:, ks, 2 * INNER:2 * INNER + 512],
                                               start=(ks == 0), stop=(ks == 5))
                              nc.tensor.matmul(vp[:, 512:768],
                                               xT[:, ks, tt * 128:(tt + 1) * 128],
                                               wq[:, ks, 2 * INNER + 512:3 * INNER],
                                               start=(ks == 0), stop=(ks == 5))
                          nc.vector.tensor_copy(
                              vhat[:, blk_i * 4 + tt, :], vp[:, 0:768])

                  for blk in range(NQ // BLK):
                      proj_block(qx, blk, True)
                  for blk in range(N // BLK):
                      proj_block(xb, blk, False)

            # ---------------- phase A: attention ----------------
            with tc.tile_pool(name="pP", bufs=6) as pP, \
                 tc.tile_pool(name="psumS", bufs=2, space="PSUM") as pS, \
                 tc.tile_pool(name="psumO", bufs=2, space="PSUM") as pO, \
                 tc.tile_pool(name="psumD", bufs=2, space="PSUM") as pD:
                # head-pair processing: heads (2hp, 2hp+1) on PE row groups
                # 0-63 / 64-127; score matmuls are concurrent K=64 row-tiles,
                # attnV matmuls are concurrent M=64 col-tiles. Softmax
                # denominators accumulate via M=1 matmuls into one PSUM bank
                # at partitions 32*(2qh+j); unwritten rows hold 1.0.
                for hp in range(6):
                    ots = {}
                    for qh in range(2):
                        ots[qh] = pO.tile([128, 512], F32, tag="pO",
                                          name=f"ot_{hp}_{qh}")
                    den_ps = pD.tile([128, 512], F32, tag="pD",
                                     name=f"den_{hp}")
                    nc.vector.memset(den_ps, 1.0)
                    for kb in range(KT):
                        kbsl = bass.ts(kb, 128)
                        sts = {}
                        for qh in range(2):
                            qsl = bass.ts(qh, 512)
                            st = pS.tile([128, 1024], F32, tag="pS",
                                         name=f"st_{qh}")
                            for j in range(2):
                                rb = j * 64
                                nc.tensor.matmul(
                                    st[:, j * 512:(j + 1) * 512],
                                    khat[rb:rb + 64, hp, kbsl],
                                    qhat[rb:rb + 64, hp, qsl],
                                    start=True, stop=True)
                            sts[qh] = st
                        for qh in range(2):
                            pt = pP.tile([128, 1024], BF16, tag="pP",
                                         name=f"pt_{qh}")
                            nc.scalar.activation(pt, sts[qh], AF.Exp)
                            for j in range(2):
                                h = 2 * hp + j
                                nc.tensor.matmul(
                                    ots[qh][j * 64:(j + 1) * 64, :],
                                    vhat[:, kb, h * DH:(h + 1) * DH],
                                    pt[:, j * 512:(j + 1) * 512],
                                    start=(kb == 0), stop=(kb == KT - 1))
                            for j in range(2):
                                r = 32 * (2 * qh + j)
                                nc.tensor.matmul(
                                    den_ps[r:r + 1, :],
                                    ones128,
                                    pt[:, j * 512:(j + 1) * 512],
                                    start=(kb == 0), stop=(kb == KT - 1),
                                    tile_position=(0, r))
                    for qh in range(2):
                        qsl = bass.ts(qh, 512)
                        nc.vector.tensor_copy(o2_all[:, hp, qsl], ots[qh])
                    with nc.allow_low_precision(
                            reason="bf16 1/den: 0.4% rel on softmax scale, "
                                   "well under the 2e-2 budget"):
                        nc.vector.reciprocal(rinv_all[:, hp, :], den_ps)

            # ---------------- normalization ----------------
            with tc.tile_pool(name="psumR", bufs=4, space="PSUM") as pR:
                for hp in range(6):
                    for qh in range(2):
                        qsl = bass.ts(qh, 512)
                        rbc = pR.tile([128, 512], F32, tag="pR",
                                      name=f"rbc_{2 * hp + qh}")
                        nc.tensor.matmul(rbc, selq_bf[:, qh, :],
                                         rinv_all[:, hp, :],
                                         start=True, stop=True)
                        nc.vector.tensor_mul(
                            o2_all[:, hp, qsl], o2_all[:, hp, qsl], rbc)

            # ---------------- phase Y: output projection ----------------
            with tc.tile_pool(name="pys", bufs=2) as pys, \
                 tc.tile_pool(name="psumY", bufs=2, space="PSUM") as pY:
                for mt in range(NQ // 128):
                    mtsl = bass.ts(mt, 128)
                    yp = pY.tile([128, 1024], F32, tag="pY")
                    for hp in range(6):
                        lhsT = o2_all[:, hp, mtsl]
                        nc.tensor.matmul(yp[:, 0:512], lhsT, wo12[:, hp, 0:512],
                                         start=(hp == 0), stop=False)
                        nc.tensor.matmul(yp[:, 512:768], lhsT, wo12[:, hp, 512:768],
                                         start=(hp == 0), stop=False)
                    nc.tensor.matmul(yp[:, 0:512], ones_bf, b_bf[:, 0:512],
                                     start=False, stop=True)
                    nc.tensor.matmul(yp[:, 512:768], ones_bf, b_bf[:, 512:768],
                                     start=False, stop=True)
                    ys = pys.tile([128, D], F32, tag="ys")
                    nc.vector.tensor_copy(ys, yp[:, 0:768])
                    nc.sync.dma_start(out=y[mt * 128:(mt + 1) * 128, :], in_=ys)

    _split_multi_waits(nc)
    return nc


_prog_cache = {}


def _make_in_maps(inputs):
    x = np.ascontiguousarray(np.asarray(inputs["x"], dtype=np.float32))
    w_qkv = np.asarray(inputs["w_qkv"], dtype=np.float32)
    w_out = np.asarray(inputs["w_out"], dtype=np.float32)
    b_out = np.asarray(inputs["b_out"], dtype=np.float32).reshape(1, D)

    wqkvT = np.ascontiguousarray(w_qkv.T)            # [768, 2304]
    woT = np.ascontiguousarray(w_out.T)              # [768, 768]
    p = np.arange(128)
    sel = (p[:, None] % 64 == p[None, :] % 64).astype(np.float32)
    # selq[r, qh, c] = 1 where r == 32*(2*qh + (c >= 64)): broadcasts the
    # 1/den rows (at partitions 0/32/64/96) to the pair-stacked layout
    r = np.arange(128)[:, None, None]
    qhi = np.arange(2)[None, :, None]
    c = np.arange(128)[None, None, :]
    selq = (r == 32 * (2 * qhi + (c >= 64))).astype(np.float32)
    selq = selq.reshape(128, 2 * 128)

    in_maps = []
    for cix in range(NCORES):
        bi, qi = cix // 4, cix % 4
        in_maps.append({
            "xb": x[bi],
            "qx": np.ascontiguousarray(x[bi, qi * NQ:(qi + 1) * NQ]),
            "wqkvT": wqkvT,
            "woT": woT,
            "bout": b_out,
            "selin": sel,
            "selqin": selq,
        })
    return in_maps


def kernel(x, w_qkv, w_out, b_out, scale):
    scale = np.asarray(scale, dtype=np.float32)
    inv_scale = tuple(float(1.0 / s) for s in scale)
    nc = _prog_cache.get(inv_scale)
    if nc is None:
        nc = _build_program(inv_scale)
        _prog_cache[inv_scale] = nc

    in_maps = _make_in_maps(
        {"x": x, "w_qkv": w_qkv, "w_out": w_out, "b_out": b_out})

    res = run_bass_kernel_spmd(nc, in_maps, core_ids=list(range(NCORES)))
    out = np.empty((B, N, D), dtype=np.float32)
    for c in range(NCORES):
        bi, qi = c // 4, c % 4
        out[bi, qi * NQ:(qi + 1) * NQ] = res.results[c]["y"]
    return out


# revision 14
# speedup vs baseline: 1.4399x; 1.4399x over previous
"""CosineSimilarityAttention Trainium2 kernel (8 NeuronCores, SPMD).

Sharding: token-parallel. Global tokens = 2 batches x 4096. Core c handles
batch (c // 4), query rows (c % 4)*1024 .. +1024. Each core computes K/V
projections for its whole batch (4096 tokens), plus Q for its own 1024
tokens, then attention and the output projection for its token slice.

Math per batch (faithful to reference):
  qkv = x @ w_qkv.T ; split q,k,v ; reshape heads h=12, dh=64
  q *= 1/sqrt(||q||_heads + eps)   (L2 norm over the HEADS axis, per (n, dh))
  k *= 1/sqrt(||k||_heads + eps)
  out_h = softmax((q_h k_h^T) / scale_h) v_h   (no max-subtract: |logits|<~2)
  y = concat_h(out_h) @ w_out.T + b_out

Perf notes (measured on this stack):
  - partial-array matmuls (K=64 row tiles / M<=64 col tiles) do NOT
    register as PE activity for the HAM clock gate and do NOT run
    concurrently here (walrus --enable-ldw-opt=false): phase A must use
    full-K zero-padded score matmuls + the 65-column attnV trick.
  - W+P is software-pipelined (transpose/drain of block b+1 is emitted
    before the norm chain of block b) so the PE never idles >3.4us and
    the clock gate stays at 8/8.
  - softmax normalization is per-head-pair: denominators staged to
    partitions {0,64} (DVE partition-base alignment), one reciprocal per
    pair, selector matmul broadcasts 1/den; runs under the next pair's
    exp wall instead of 24 flat-cost 3.3us reciprocals on the tail.
  - output projection contracts head PAIRS (K=128) from the pair-stacked
    attention-output layout.
"""

import numpy as np

import concourse.bass as bass
import concourse.mybir as mybir
import concourse.tile as tile
from concourse.bass_utils import run_bass_kernel_spmd
from concourse.masks import make_identity

F32 = mybir.dt.float32
BF16 = mybir.dt.bfloat16
AF = mybir.ActivationFunctionType

B = 2
N = 4096          # tokens per batch
D = 768           # model dim
H = 12            # heads
DH = 64           # head dim
INNER = H * DH    # 768
EPS = 1e-8
NQ = 1024         # query tokens per core
NCORES = 8
BLK = 512         # projection token block
KT = N // 128     # 32 key tiles of 128


def _split_multi_waits(nc):
    """This container's walrus accepts only ONE sync-wait per instruction.
    Hoist extra waits into standalone EVSEM instructions placed just before."""
    n = 0
    for f in nc.m.functions:
        for bb in f.blocks:
            insts = list(bb.instructions)
            out = []
            for inst in insts:
                si = inst.sync_info
                if si is not None and si.on_wait is not None and len(si.on_wait) > 1:
                    waits = list(si.on_wait)
                    for j, w in enumerate(waits[:-1]):
                        ev = mybir.InstEventSemaphore(
                            name=f"{inst.name}-evw{j}",
                            engine=inst.engine,
                            sync_info=mybir.SyncInfo(on_wait=[w], on_update=[]),
                        )
                        out.append(ev)
                        n += 1
                    si.on_wait = [waits[-1]]
                out.append(inst)
            bb.instructions = out
    return n


def _build_program(inv_scale):
    """Build the single SPMD Bass program. inv_scale: list of 12 floats."""
    nc = bass.Bass()
    xb = nc.declare_dram_parameter("xb", [N, D], F32, isOutput=False)
    qx = nc.declare_dram_parameter("qx", [NQ, D], F32, isOutput=False)
    wqkvT = nc.declare_dram_parameter("wqkvT", [D, 3 * INNER], F32, isOutput=False)
    woT = nc.declare_dram_parameter("woT", [INNER, D], F32, isOutput=False)
    bout = nc.declare_dram_parameter("bout", [1, D], F32, isOutput=False)
    selin = nc.declare_dram_parameter("selin", [128, 128], F32, isOutput=False)
    selqin = nc.declare_dram_parameter("selqin", [128, 128], F32, isOutput=False)
    y = nc.declare_dram_parameter("y", [NQ, D], F32, isOutput=True)

    with tile.TileContext(nc) as tc:
        with tc.tile_pool(name="const", bufs=1) as constp, \
             tc.tile_pool(name="persist", bufs=1) as persist:
            # --- constants ---
            ident = constp.tile([128, 128], F32)
            make_identity(nc, ident)
            sel_bf = constp.tile([128, 128], BF16)
            selq_bf = constp.tile([128, 128], BF16)
            ones_f = constp.tile([1, 64], F32)
            nc.vector.memset(ones_f, 1.0)
            ones_bf = constp.tile([1, 128], BF16)
            nc.vector.memset(ones_bf, 1.0)
            eps_t = constp.tile([128, 1], F32)
            nc.vector.memset(eps_t, EPS)
            invs = constp.tile([128, 6], F32)
            for dt in range(6):
                nc.vector.memset(invs[0:64, dt:dt + 1], float(inv_scale[2 * dt]))
                nc.vector.memset(invs[64:128, dt:dt + 1],
                                 float(inv_scale[2 * dt + 1]))
            b_bf = constp.tile([1, D], BF16)
            with tc.tile_pool(name="cstage", bufs=1) as cst:
                sel_st = cst.tile([128, 128], F32, tag="sel")
                nc.sync.dma_start(out=sel_st, in_=selin[:, :])
                nc.vector.tensor_copy(sel_bf, sel_st)
                selq_st = cst.tile([128, 128], F32, tag="sel")
                nc.sync.dma_start(out=selq_st, in_=selqin[:, :])
                nc.vector.tensor_copy(selq_bf, selq_st)
                b_st = cst.tile([1, D], F32, tag="b")
                nc.sync.dma_start(out=b_st, in_=bout[:, :])
                nc.vector.tensor_copy(b_bf, b_st)

            # --- persistent activations ---
            khat = persist.tile([128, 6, N], BF16)     # k^T normed [dim, tok]
            qhat = persist.tile([128, H, NQ], BF16)    # q^T per head, zero-pad
            vhat = persist.tile([128, KT, H * 65], BF16)  # v [tok, h*65] (+ones)
            o2_all = persist.tile([128, 6, NQ], BF16)  # attn out, pair-stacked
            wo12 = persist.tile([128, 6, D], BF16)     # w_out pair-stacked

            # ones columns of vhat (col 64 of every 65-block)
            vones = vhat.rearrange("p t (h c) -> p t h c", c=65)[:, :, :, 64:65]
            nc.vector.memset(vones, 1.0)
            nc.vector.memset(qhat, 0.0)

            # output-projection weights: head h rows -> partitions (h%2)*64
            with tc.tile_pool(name="wostage", bufs=2) as wost:
                for h in range(H):
                    wst_t = wost.tile([64, D], F32, tag="wost")
                    nc.sync.dma_start(out=wst_t, in_=woT[h * 64:(h + 1) * 64, :])
                    base = (h % 2) * 64
                    nc.vector.tensor_copy(wo12[base:base + 64, h // 2, :], wst_t)

            # ---------------- phase W+P: weights, projections, head-norm ----
            with tc.tile_pool(name="pw", bufs=1) as pwp:
              wq = pwp.tile([128, 6, 3 * INNER], BF16)
              with tc.tile_pool(name="wstage", bufs=1) as wst:
                for dt in range(6):
                    st = wst.tile([128, 3 * INNER], F32, tag="wst")
                    nc.sync.dma_start(out=st, in_=wqkvT[dt * 128:(dt + 1) * 128, :])
                    nc.vector.tensor_copy(wq[:, dt, :], st)
              with tc.tile_pool(name="pstage", bufs=2) as pstage, \
                   tc.tile_pool(name="pxT", bufs=2) as pxT, \
                   tc.tile_pool(name="pkf", bufs=1) as pkf, \
                   tc.tile_pool(name="pksq", bufs=2) as pksq, \
                   tc.tile_pool(name="psmall", bufs=1) as psmall, \
                   tc.tile_pool(name="psumTP", bufs=1, space="PSUM") as pTP, \
                   tc.tile_pool(name="psumKP", bufs=2, space="PSUM") as pKP, \
                   tc.tile_pool(name="psumSQ", bufs=1, space="PSUM") as pSQ, \
                   tc.tile_pool(name="psumVP", bufs=2, space="PSUM") as pVP:

                  def load_transpose(src, blk_i):
                      # DMA x block [512, D] + transpose -> xT [dim, tok] bf16.
                      # Emitted one block ahead of the compute part so the
                      # PE's projection matmuls never wait on the DVE drains.
                      xsts = []
                      for hh in range(2):
                          xst = pstage.tile([128, 2, D], F32, tag="xst")
                          nc.sync.dma_start(
                              out=xst,
                              in_=src[blk_i * BLK + hh * 256:
                                      blk_i * BLK + (hh + 1) * 256, :].rearrange(
                                  "(t p) d -> p t d", p=128),
                          )
                          xsts.append(xst)
                      xT = pxT.tile([128, 6, BLK], BF16, tag="xT")
                      for dt in range(6):
                          tp = pTP.tile([128, 512], F32, tag="pTP")
                          for tt in range(4):
                              nc.tensor.transpose(
                                  tp[:, tt * 128:(tt + 1) * 128],
                                  xsts[tt // 2][:, tt % 2,
                                                dt * 128:(dt + 1) * 128], ident)
                          nc.vector.tensor_copy(xT[:, dt, :], tp)
                      return xT

                  def proj_compute(xT, blk_i, is_q):
                      wbase = 0 if is_q else INNER
                      # q^T / k^T projection [dim_out, tok]; kf bf16 via
                      # ScalarE, squares straight from PSUM via ScalarE Square
                      kf = pkf.tile([128, 6, BLK], BF16, tag="kf")
                      sq = pSQ.tile([128, 512], F32, tag="pSQ")
                      for dt in range(6):
                          kp = pKP.tile([128, 512], F32, tag="pKP")
                          for ks in range(6):
                              nc.tensor.matmul(
                                  kp,
                                  wq[:, ks, wbase + dt * 128: wbase + (dt + 1) * 128],
                                  xT[:, ks, :],
                                  start=(ks == 0), stop=(ks == 5))
                          nc.scalar.copy(kf[:, dt, :], kp)
                          ksq = pksq.tile([128, BLK], BF16, tag="ksq")
                          nc.scalar.square(ksq, kp)
                          nc.tensor.matmul(sq, sel_bf, ksq,
                                           start=(dt == 0), stop=(dt == 5))
                      nrm = psmall.tile([128, BLK], F32, tag="nrm")
                      nc.scalar.activation(nrm, sq, AF.Sqrt)
                      u = psmall.tile([128, BLK], F32, tag="u")
                      nc.scalar.activation(u, nrm, AF.Sqrt, bias=eps_t[:, :])
                      rq = psmall.tile([128, BLK], F32, tag="nrm")
                      nc.vector.reciprocal(rq, u)
                      bsl = bass.ts(blk_i, BLK)
                      if is_q:
                          # zero-padded per-head layout: head 2dt on rows 0:64,
                          # head 2dt+1 on rows 64:128, other rows stay zero.
                          # 1/scale_h is folded in so exp needs no scale.
                          for dt in range(6):
                              a = qhat[0:64, 2 * dt, bsl]
                              b = qhat[64:128, 2 * dt + 1, bsl]
                              nc.vector.tensor_mul(a, kf[0:64, dt, :], rq[0:64, :])
                              nc.vector.tensor_mul(b, kf[64:128, dt, :],
                                                   rq[64:128, :])
                              nc.vector.tensor_scalar_mul(a, a,
                                                          invs[0:64, dt:dt + 1])
                              nc.vector.tensor_scalar_mul(b, b,
                                                          invs[64:128, dt:dt + 1])
                          return
                      for dt in range(6):
                          nc.vector.tensor_mul(
                              khat[:, dt, bsl], kf[:, dt, :], rq)
                      # v projection [tok, inner] -> vhat strided 65
                      for tt in range(4):
                          vp = pVP.tile([128, 1024], F32, tag="pVP")
                          for ks in range(6):
                              nc.tensor.matmul(vp[:, 0:512],
                                               xT[:, ks, tt * 128:(tt + 1) * 128],
                                               wq[:, ks, 2 * INNER:2 * INNER + 512],
                                               start=(ks == 0), stop=(ks == 5))
                              nc.tensor.matmul(vp[:, 512:768],
                                               xT[:, ks, tt * 128:(tt + 1) * 128],
                                               wq[:, ks, 2 * INNER + 512:3 * INNER],
                                               start=(ks == 0), stop=(ks == 5))
                          vdst = vhat[:, blk_i * 4 + tt, :].rearrange(
                              "p (h c) -> p h c", c=65)[:, :, 0:64]
                          nc.vector.tensor_copy(
                              vdst, vp[:, 0:768].rearrange("p (h c) -> p h c", c=64))

                  # software pipeline: transpose part of block i+1 lands in
                  # the engine queues before the norm chain of block i
                  blocks = [(qx, b, True) for b in range(NQ // BLK)] + \
                           [(xb, b, False) for b in range(N // BLK)]
                  xT_pend = load_transpose(blocks[0][0], blocks[0][1])
                  for i, (src, b, is_q) in enumerate(blocks):
                      xT_cur = xT_pend
                      if i + 1 < len(blocks):
                          nsrc, nb, _ = blocks[i + 1]
                          xT_pend = load_transpose(nsrc, nb)
                      proj_compute(xT_cur, b, is_q)

            # ---------------- phase A: attention ----------------
            with tc.tile_pool(name="pP", bufs=6) as pP, \
                 tc.tile_pool(name="pden", bufs=2) as pden, \
                 tc.tile_pool(name="princ", bufs=2) as princ, \
                 tc.tile_pool(name="psumS", bufs=2, space="PSUM") as pS, \
                 tc.tile_pool(name="psumO", bufs=4, space="PSUM") as pO:
                # head-pair processing: heads (2hp, 2hp+1) live on PE row
                # groups 0-63 / 64-127 via the zero-padded qhat. Queries in
                # 512-halves so every PSUM tile is one bank.
                for hp in range(6):
                    ots = {}
                    for j in range(2):
                        for qh in range(2):
                            ots[(j, qh)] = pO.tile([65, 512], F32, tag="pO",
                                                   name=f"ot_{j}_{qh}")
                    for kb in range(KT):
                        kbsl = bass.ts(kb, 128)
                        sts = {}
                        for qh in range(2):
                            qsl = bass.ts(qh, 512)
                            st = pS.tile([128, 1024], F32, tag="pS",
                                         name=f"st_{qh}")
                            for j in range(2):
                                nc.tensor.matmul(
                                    st[:, j * 512:(j + 1) * 512],
                                    khat[:, hp, kbsl],
                                    qhat[:, 2 * hp + j, qsl],
                                    start=True, stop=True)
                            sts[qh] = st
                        for qh in range(2):
                            pt = pP.tile([128, 1024], BF16, tag="pP",
                                         name=f"pt_{qh}")
                            nc.scalar.activation(pt, sts[qh], AF.Exp)
                            for j in range(2):
                                h = 2 * hp + j
                                nc.tensor.matmul(
                                    ots[(j, qh)],
                                    vhat[:, kb, h * 65:(h + 1) * 65],
                                    pt[:, j * 512:(j + 1) * 512],
                                    start=(kb == 0), stop=(kb == KT - 1))
                    # drain + normalize this pair (overlaps next pair's exp
                    # wall): denominators staged to partitions {0, 64},
                    # one reciprocal, selector matmul broadcasts 1/den
                    den_st = pden.tile([128, 1024], F32, tag="den",
                                       name=f"den_{hp}")
                    nc.vector.memset(den_st, 1.0)
                    for j in range(2):
                        for qh in range(2):
                            qsl = bass.ts(qh, 512)
                            nc.vector.tensor_copy(
                                o2_all[64 * j:64 * j + 64, hp, qsl],
                                ots[(j, qh)][0:64, :])
                            nc.vector.tensor_copy(
                                den_st[64 * j:64 * j + 1, qsl],
                                ots[(j, qh)][64:65, :])
                    rinv = princ.tile([128, 1024], BF16, tag="rinv",
                                      name=f"rinv_{hp}")
                    with nc.allow_low_precision(
                            reason="bf16 1/den: 0.4% rel on softmax scale, "
                                   "well under the 2e-2 budget"):
                        nc.vector.reciprocal(rinv, den_st)
                    for qh in range(2):
                        qsl = bass.ts(qh, 512)
                        rbc = pS.tile([128, 512], F32, tag="pS",
                                      name=f"rbc_{qh}")
                        nc.tensor.matmul(rbc, selq_bf, rinv[:, qsl],
                                         start=True, stop=True)
                        nc.vector.tensor_mul(
                            o2_all[:, hp, qsl], o2_all[:, hp, qsl], rbc)

            # ---------------- phase Y: output projection ----------------
            with tc.tile_pool(name="pys", bufs=2) as pys, \
                 tc.tile_pool(name="psumY", bufs=2, space="PSUM") as pY:
                for mt in range(NQ // 128):
                    mtsl = bass.ts(mt, 128)
                    yp = pY.tile([128, 1024], F32, tag="pY")
                    for hp in range(6):
                        lhsT = o2_all[:, hp, mtsl]
                        nc.tensor.matmul(yp[:, 0:512], lhsT, wo12[:, hp, 0:512],
                                         start=(hp == 0), stop=False)
                        nc.tensor.matmul(yp[:, 512:768], lhsT, wo12[:, hp, 512:768],
                                         start=(hp == 0), stop=False)
                    nc.tensor.matmul(yp[:, 0:512], ones_bf, b_bf[:, 0:512],
                                     start=False, stop=True)
                    nc.tensor.matmul(yp[:, 512:768], ones_bf, b_bf[:, 512:768],
                                     start=False, stop=True)
                    ys = pys.tile([128, D], F32, tag="ys")
                    nc.vector.tensor_copy(ys, yp[:, 0:768])
                    nc.sync.dma_start(out=y[mt * 128:(mt + 1) * 128, :], in_=ys)

    _split_multi_waits(nc)
    return nc


_prog_cache = {}


def _make_in_maps(inputs):
    x = np.ascontiguousarray(np.asarray(inputs["x"], dtype=np.float32))
    w_qkv = np.asarray(inputs["w_qkv"], dtype=np.float32)
    w_out = np.asarray(inputs["w_out"], dtype=np.float32)
    b_out = np.asarray(inputs["b_out"], dtype=np.float32).reshape(1, D)

    wqkvT = np.ascontiguousarray(w_qkv.T)            # [768, 2304]
    woT = np.ascontiguousarray(w_out.T)              # [768, 768]
    p = np.arange(128)
    sel = (p[:, None] % 64 == p[None, :] % 64).astype(np.float32)
    # selq[r, c] = 1 where r == 64*(c >= 64): broadcasts the 1/den rows
    # (staged at partitions 0/64) to the pair-stacked [128, q] layout
    selq = (p[:, None] == 64 * (p[None, :] >= 64)).astype(np.float32)

    in_maps = []
    for cix in range(NCORES):
        bi, qi = cix // 4, cix % 4
        in_maps.append({
            "xb": x[bi],
            "qx": np.ascontiguousarray(x[bi, qi * NQ:(qi + 1) * NQ]),
            "wqkvT": wqkvT,
            "woT": woT,
            "bout": b_out,
            "selin": sel,
            "selqin": selq,
        })
    return in_maps


def kernel(x, w_qkv, w_out, b_out, scale):
    scale = np.asarray(scale, dtype=np.float32)
    inv_scale = tuple(float(1.0 / s) for s in scale)
    nc = _prog_cache.get(inv_scale)
    if nc is None:
        nc = _build_program(inv_scale)
        _prog_cache[inv_scale] = nc

    in_maps = _make_in_maps(
        {"x": x, "w_qkv": w_qkv, "w_out": w_out, "b_out": b_out})

    res = run_bass_kernel_spmd(nc, in_maps, core_ids=list(range(NCORES)))
    out = np.empty((B, N, D), dtype=np.float32)
    for c in range(NCORES):
        bi, qi = c // 4, c % 4
        out[bi, qi * NQ:(qi + 1) * NQ] = res.results[c]["y"]
    return out


# revision 15
# speedup vs baseline: 1.5707x; 1.0908x over previous
"""CosineSimilarityAttention Trainium2 kernel (8 NeuronCores, SPMD).

Sharding: token-parallel. Global tokens = 2 batches x 4096. Core c handles
batch (c // 4), query rows (c % 4)*1024 .. +1024. Each core computes K/V
projections for its whole batch (4096 tokens), plus Q for its own 1024
tokens, then attention and the output projection for its token slice.

Math per batch (faithful to reference):
  qkv = x @ w_qkv.T ; split q,k,v ; reshape heads h=12, dh=64
  q *= 1/sqrt(||q||_heads + eps)   (L2 norm over the HEADS axis, per (n, dh))
  k *= 1/sqrt(||k||_heads + eps)
  out_h = softmax((q_h k_h^T) / scale_h) v_h   (no max-subtract: |logits|<~2)
  y = concat_h(out_h) @ w_out.T + b_out

Perf notes (measured on this stack):
  - partial-array matmuls (K=64 row tiles / M<=64 col tiles) do NOT
    register as PE activity for the HAM clock gate and do NOT run
    concurrently here (walrus --enable-ldw-opt=false): phase A must use
    full-K zero-padded score matmuls + the 65-column attnV trick.
  - W+P is software-pipelined (transpose/drain of block b+1 is emitted
    before the norm chain of block b) so the PE never idles >3.4us and
    the clock gate stays at 8/8.
  - softmax normalization is per-head-pair: denominators staged to
    partitions {0,64} (DVE partition-base alignment), one reciprocal per
    pair, selector matmul broadcasts 1/den; runs under the next pair's
    exp wall instead of 24 flat-cost 3.3us reciprocals on the tail.
  - output projection contracts head PAIRS (K=128) from the pair-stacked
    attention-output layout.
"""

import numpy as np

import concourse.bass as bass
import concourse.mybir as mybir
import concourse.tile as tile
from concourse.bass_utils import run_bass_kernel_spmd
from concourse.masks import make_identity

F32 = mybir.dt.float32
BF16 = mybir.dt.bfloat16
AF = mybir.ActivationFunctionType

B = 2
N = 4096          # tokens per batch
D = 768           # model dim
H = 12            # heads
DH = 64           # head dim
INNER = H * DH    # 768
EPS = 1e-8
NQ = 1024         # query tokens per core
NCORES = 8
BLK = 512         # projection token block
KT = N // 128     # 32 key tiles of 128


def _split_multi_waits(nc):
    """This container's walrus accepts only ONE sync-wait per instruction.
    Hoist extra waits into standalone EVSEM instructions placed just before."""
    n = 0
    for f in nc.m.functions:
        for bb in f.blocks:
            insts = list(bb.instructions)
            out = []
            for inst in insts:
                si = inst.sync_info
                if si is not None and si.on_wait is not None and len(si.on_wait) > 1:
                    waits = list(si.on_wait)
                    for j, w in enumerate(waits[:-1]):
                        ev = mybir.InstEventSemaphore(
                            name=f"{inst.name}-evw{j}",
                            engine=inst.engine,
                            sync_info=mybir.SyncInfo(on_wait=[w], on_update=[]),
                        )
                        out.append(ev)
                        n += 1
                    si.on_wait = [waits[-1]]
                out.append(inst)
            bb.instructions = out
    return n


def _build_program(inv_scale):
    """Build the single SPMD Bass program. inv_scale: list of 12 floats."""
    nc = bass.Bass()
    xb = nc.declare_dram_parameter("xb", [N, D], F32, isOutput=False)
    qx = nc.declare_dram_parameter("qx", [NQ, D], F32, isOutput=False)
    wqkvT = nc.declare_dram_parameter("wqkvT", [D, 3 * INNER], F32, isOutput=False)
    woT = nc.declare_dram_parameter("woT", [INNER, D], F32, isOutput=False)
    bout = nc.declare_dram_parameter("bout", [1, D], F32, isOutput=False)
    selin = nc.declare_dram_parameter("selin", [128, 128], F32, isOutput=False)
    selqin = nc.declare_dram_parameter("selqin", [128, 128], F32, isOutput=False)
    y = nc.declare_dram_parameter("y", [NQ, D], F32, isOutput=True)

    with tile.TileContext(nc) as tc:
        with tc.tile_pool(name="const", bufs=1) as constp, \
             tc.tile_pool(name="persist", bufs=1) as persist:
            # --- constants ---
            ident = constp.tile([128, 128], F32)
            make_identity(nc, ident)
            sel_bf = constp.tile([128, 128], BF16)
            selq_bf = constp.tile([128, 128], BF16)
            ones_f = constp.tile([1, 64], F32)
            nc.vector.memset(ones_f, 1.0)
            ones_bf = constp.tile([1, 128], BF16)
            nc.vector.memset(ones_bf, 1.0)
            eps_t = constp.tile([128, 1], F32)
            nc.vector.memset(eps_t, EPS)
            invs = constp.tile([128, 6], F32)
            for dt in range(6):
                nc.vector.memset(invs[0:64, dt:dt + 1], float(inv_scale[2 * dt]))
                nc.vector.memset(invs[64:128, dt:dt + 1],
                                 float(inv_scale[2 * dt + 1]))
            b_bf = constp.tile([1, D], BF16)
            with tc.tile_pool(name="cstage", bufs=1) as cst:
                sel_st = cst.tile([128, 128], F32, tag="sel")
                nc.sync.dma_start(out=sel_st, in_=selin[:, :])
                nc.vector.tensor_copy(sel_bf, sel_st)

            # --- persistent activations ---
            khat = persist.tile([128, 6, N], BF16)     # k^T normed [dim, tok]
            qhat = persist.tile([128, H, NQ], BF16)    # q^T per head, zero-pad
            vhat = persist.tile([128, KT, H * 65], BF16)  # v [tok, h*65] (+ones)
            o2_all = persist.tile([128, 6, NQ], BF16)  # attn out, pair-stacked
            wo12 = persist.tile([128, 6, D], BF16)     # w_out pair-stacked

            # ---------------- phase W+P: weights, projections, head-norm ----
            with tc.tile_pool(name="pw", bufs=1) as pwp:
              wq = pwp.tile([128, 6, 3 * INNER], BF16)
              with tc.tile_pool(name="wstage", bufs=1) as wst:
                for dt in range(6):
                    st = wst.tile([128, 3 * INNER], F32, tag="wst")
                    nc.sync.dma_start(out=st, in_=wqkvT[dt * 128:(dt + 1) * 128, :])
                    nc.vector.tensor_copy(wq[:, dt, :], st)
              with tc.tile_pool(name="pstage", bufs=2) as pstage, \
                   tc.tile_pool(name="pxT", bufs=2) as pxT, \
                   tc.tile_pool(name="pkf", bufs=1) as pkf, \
                   tc.tile_pool(name="pksq", bufs=2) as pksq, \
                   tc.tile_pool(name="psmall", bufs=1) as psmall, \
                   tc.tile_pool(name="psumTP", bufs=1, space="PSUM") as pTP, \
                   tc.tile_pool(name="psumKP", bufs=2, space="PSUM") as pKP, \
                   tc.tile_pool(name="psumSQ", bufs=1, space="PSUM") as pSQ, \
                   tc.tile_pool(name="psumVP", bufs=2, space="PSUM") as pVP:

                  def load_transpose(src, blk_i):
                      # DMA x block [512, D] + transpose -> xT [dim, tok] bf16.
                      # Emitted one block ahead of the compute part so the
                      # PE's projection matmuls never wait on the DVE drains.
                      xsts = []
                      for hh in range(2):
                          xst = pstage.tile([128, 2, D], F32, tag="xst")
                          nc.sync.dma_start(
                              out=xst,
                              in_=src[blk_i * BLK + hh * 256:
                                      blk_i * BLK + (hh + 1) * 256, :].rearrange(
                                  "(t p) d -> p t d", p=128),
                          )
                          xsts.append(xst)
                      xT = pxT.tile([128, 6, BLK], BF16, tag="xT")
                      for dt in range(6):
                          tp = pTP.tile([128, 512], F32, tag="pTP")
                          for tt in range(4):
                              nc.tensor.transpose(
                                  tp[:, tt * 128:(tt + 1) * 128],
                                  xsts[tt // 2][:, tt % 2,
                                                dt * 128:(dt + 1) * 128], ident)
                          nc.vector.tensor_copy(xT[:, dt, :], tp)
                      return xT

                  def proj_compute(xT, blk_i, is_q):
                      wbase = 0 if is_q else INNER
                      # q^T / k^T projection [dim_out, tok]; kf bf16 via
                      # ScalarE, squares straight from PSUM via ScalarE Square
                      kf = pkf.tile([128, 6, BLK], BF16, tag="kf")
                      sq = pSQ.tile([128, 512], F32, tag="pSQ")
                      for dt in range(6):
                          kp = pKP.tile([128, 512], F32, tag="pKP")
                          for ks in range(6):
                              nc.tensor.matmul(
                                  kp,
                                  wq[:, ks, wbase + dt * 128: wbase + (dt + 1) * 128],
                                  xT[:, ks, :],
                                  start=(ks == 0), stop=(ks == 5))
                          nc.scalar.copy(kf[:, dt, :], kp)
                          ksq = pksq.tile([128, BLK], BF16, tag="ksq")
                          nc.scalar.square(ksq, kp)
                          nc.tensor.matmul(sq, sel_bf, ksq,
                                           start=(dt == 0), stop=(dt == 5))
                      nrm = psmall.tile([128, BLK], F32, tag="nrm")
                      nc.scalar.activation(nrm, sq, AF.Sqrt)
                      u = psmall.tile([128, BLK], F32, tag="u")
                      nc.scalar.activation(u, nrm, AF.Sqrt, bias=eps_t[:, :])
                      rq = psmall.tile([128, BLK], F32, tag="nrm")
                      nc.vector.reciprocal(rq, u)
                      bsl = bass.ts(blk_i, BLK)
                      if is_q:
                          # zero-padded per-head layout: head 2dt on rows 0:64,
                          # head 2dt+1 on rows 64:128, other rows stay zero.
                          # 1/scale_h is folded in so exp needs no scale.
                          for dt in range(6):
                              a = qhat[0:64, 2 * dt, bsl]
                              b = qhat[64:128, 2 * dt + 1, bsl]
                              nc.vector.tensor_mul(a, kf[0:64, dt, :], rq[0:64, :])
                              nc.vector.tensor_mul(b, kf[64:128, dt, :],
                                                   rq[64:128, :])
                              nc.vector.tensor_scalar_mul(a, a,
                                                          invs[0:64, dt:dt + 1])
                              nc.vector.tensor_scalar_mul(b, b,
                                                          invs[64:128, dt:dt + 1])
                          return
                      for dt in range(6):
                          nc.vector.tensor_mul(
                              khat[:, dt, bsl], kf[:, dt, :], rq)
                      # v projection [tok, inner] -> vhat strided 65
                      for tt in range(4):
                          vp = pVP.tile([128, 1024], F32, tag="pVP")
                          for ks in range(6):
                              nc.tensor.matmul(vp[:, 0:512],
                                               xT[:, ks, tt * 128:(tt + 1) * 128],
                                               wq[:, ks, 2 * INNER:2 * INNER + 512],
                                               start=(ks == 0), stop=(ks == 5))
                              nc.tensor.matmul(vp[:, 512:768],
                                               xT[:, ks, tt * 128:(tt + 1) * 128],
                                               wq[:, ks, 2 * INNER + 512:3 * INNER],
                                               start=(ks == 0), stop=(ks == 5))
                          vdst = vhat[:, blk_i * 4 + tt, :].rearrange(
                              "p (h c) -> p h c", c=65)[:, :, 0:64]
                          nc.vector.tensor_copy(
                              vdst, vp[:, 0:768].rearrange("p (h c) -> p h c", c=64))

                  # software pipeline: transpose part of block i+1 lands in
                  # the engine queues before the norm chain of block i
                  blocks = [(qx, b, True) for b in range(NQ // BLK)] + \
                           [(xb, b, False) for b in range(N // BLK)]
                  xT_pend = load_transpose(blocks[0][0], blocks[0][1])
                  # memsets after the first block's DMA+drains are queued
                  vones = vhat.rearrange(
                      "p t (h c) -> p t h c", c=65)[:, :, :, 64:65]
                  nc.vector.memset(vones, 1.0)
                  nc.vector.memset(qhat, 0.0)
                  for i, (src, b, is_q) in enumerate(blocks):
                      xT_cur = xT_pend
                      if i + 1 < len(blocks):
                          nsrc, nb, _ = blocks[i + 1]
                          xT_pend = load_transpose(nsrc, nb)
                      proj_compute(xT_cur, b, is_q)

            # late staging: output-projection weights (head h rows ->
            # partitions (h%2)*64), normalization selector, bias. Emitted
            # here so their DMAs/casts don't delay the W+P start; they
            # execute under phase A's exp wall.
            with tc.tile_pool(name="wostage", bufs=2) as wost:
                for h in range(H):
                    wst_t = wost.tile([64, D], F32, tag="wost")
                    nc.sync.dma_start(out=wst_t, in_=woT[h * 64:(h + 1) * 64, :])
                    base = (h % 2) * 64
                    nc.vector.tensor_copy(wo12[base:base + 64, h // 2, :], wst_t)
                selq_st = wost.tile([128, 128], F32, tag="selq")
                nc.sync.dma_start(out=selq_st, in_=selqin[:, :])
                nc.vector.tensor_copy(selq_bf, selq_st)
                b_st = wost.tile([1, D], F32, tag="b")
                nc.sync.dma_start(out=b_st, in_=bout[:, :])
                nc.vector.tensor_copy(b_bf, b_st)

            # ---------------- phase A: attention ----------------
            with tc.tile_pool(name="pP", bufs=6) as pP, \
                 tc.tile_pool(name="pden", bufs=2) as pden, \
                 tc.tile_pool(name="princ", bufs=2) as princ, \
                 tc.tile_pool(name="psumS", bufs=2, space="PSUM") as pS, \
                 tc.tile_pool(name="psumO", bufs=4, space="PSUM") as pO:
                # head-pair processing: heads (2hp, 2hp+1) live on PE row
                # groups 0-63 / 64-127 via the zero-padded qhat. Queries in
                # 512-halves so every PSUM tile is one bank.
                def norm_pair(hp, ots):
                    # drain + normalize pair hp: denominators staged to
                    # partitions {0, 64}, one reciprocal, selector matmul
                    # broadcasts 1/den. Deferred into the NEXT pair's kb
                    # stream so the PE queue never idles on the DVE chain.
                    den_st = pden.tile([128, 1024], F32, tag="den",
                                       name=f"den_{hp}")
                    nc.vector.memset(den_st, 1.0)
                    for j in range(2):
                        for qh in range(2):
                            qsl = bass.ts(qh, 512)
                            nc.vector.tensor_copy(
                                o2_all[64 * j:64 * j + 64, hp, qsl],
                                ots[(j, qh)][0:64, :])
                            nc.vector.tensor_copy(
                                den_st[64 * j:64 * j + 1, qsl],
                                ots[(j, qh)][64:65, :])
                    rinv = princ.tile([128, 1024], BF16, tag="rinv",
                                      name=f"rinv_{hp}")
                    with nc.allow_low_precision(
                            reason="bf16 1/den: 0.4% rel on softmax scale, "
                                   "well under the 2e-2 budget"):
                        nc.vector.reciprocal(rinv, den_st)
                    for qh in range(2):
                        qsl = bass.ts(qh, 512)
                        rbc = pS.tile([128, 512], F32, tag="pS",
                                      name=f"rbc_{qh}")
                        nc.tensor.matmul(rbc, selq_bf, rinv[:, qsl],
                                         start=True, stop=True)
                        nc.vector.tensor_mul(
                            o2_all[:, hp, qsl], o2_all[:, hp, qsl], rbc)

                pending = None
                for hp in range(6):
                    ots = {}
                    for j in range(2):
                        for qh in range(2):
                            ots[(j, qh)] = pO.tile([65, 512], F32, tag="pO",
                                                   name=f"ot_{hp}_{j}_{qh}")
                    for kb in range(KT):
                        kbsl = bass.ts(kb, 128)
                        sts = {}
                        for qh in range(2):
                            qsl = bass.ts(qh, 512)
                            st = pS.tile([128, 1024], F32, tag="pS",
                                         name=f"st_{qh}")
                            for j in range(2):
                                nc.tensor.matmul(
                                    st[:, j * 512:(j + 1) * 512],
                                    khat[:, hp, kbsl],
                                    qhat[:, 2 * hp + j, qsl],
                                    start=True, stop=True)
                            sts[qh] = st
                        for qh in range(2):
                            pt = pP.tile([128, 1024], BF16, tag="pP",
                                         name=f"pt_{qh}")
                            nc.scalar.activation(pt, sts[qh], AF.Exp)
                            for j in range(2):
                                h = 2 * hp + j
                                nc.tensor.matmul(
                                    ots[(j, qh)],
                                    vhat[:, kb, h * 65:(h + 1) * 65],
                                    pt[:, j * 512:(j + 1) * 512],
                                    start=(kb == 0), stop=(kb == KT - 1))
                        if kb == 6 and pending is not None:
                            pending()
                            pending = None
                    pending = (lambda hp=hp, ots=ots: norm_pair(hp, ots))
                pending()

            # ---------------- phase Y: output projection ----------------
            with tc.tile_pool(name="pys", bufs=2) as pys, \
                 tc.tile_pool(name="psumY", bufs=2, space="PSUM") as pY:
                for mt in range(NQ // 128):
                    mtsl = bass.ts(mt, 128)
                    yp = pY.tile([128, 1024], F32, tag="pY")
                    for hp in range(6):
                        lhsT = o2_all[:, hp, mtsl]
                        nc.tensor.matmul(yp[:, 0:512], lhsT, wo12[:, hp, 0:512],
                                         start=(hp == 0), stop=False)
                        nc.tensor.matmul(yp[:, 512:768], lhsT, wo12[:, hp, 512:768],
                                         start=(hp == 0), stop=False)
                    nc.tensor.matmul(yp[:, 0:512], ones_bf, b_bf[:, 0:512],
                                     start=False, stop=True)
                    nc.tensor.matmul(yp[:, 512:768], ones_bf, b_bf[:, 512:768],
                                     start=False, stop=True)
                    ys = pys.tile([128, D], F32, tag="ys")
                    nc.vector.tensor_copy(ys, yp[:, 0:768])
                    nc.sync.dma_start(out=y[mt * 128:(mt + 1) * 128, :], in_=ys)

    _split_multi_waits(nc)
    return nc


_prog_cache = {}


def _make_in_maps(inputs):
    x = np.ascontiguousarray(np.asarray(inputs["x"], dtype=np.float32))
    w_qkv = np.asarray(inputs["w_qkv"], dtype=np.float32)
    w_out = np.asarray(inputs["w_out"], dtype=np.float32)
    b_out = np.asarray(inputs["b_out"], dtype=np.float32).reshape(1, D)

    wqkvT = np.ascontiguousarray(w_qkv.T)            # [768, 2304]
    woT = np.ascontiguousarray(w_out.T)              # [768, 768]
    p = np.arange(128)
    sel = (p[:, None] % 64 == p[None, :] % 64).astype(np.float32)
    # selq[r, c] = 1 where r == 64*(c >= 64): broadcasts the 1/den rows
    # (staged at partitions 0/64) to the pair-stacked [128, q] layout
    selq = (p[:, None] == 64 * (p[None, :] >= 64)).astype(np.float32)

    in_maps = []
    for cix in range(NCORES):
        bi, qi = cix // 4, cix % 4
        in_maps.append({
            "xb": x[bi],
            "qx": np.ascontiguousarray(x[bi, qi * NQ:(qi + 1) * NQ]),
            "wqkvT": wqkvT,
            "woT": woT,
            "bout": b_out,
            "selin": sel,
            "selqin": selq,
        })
    return in_maps


def kernel(x, w_qkv, w_out, b_out, scale):
    scale = np.asarray(scale, dtype=np.float32)
    inv_scale = tuple(float(1.0 / s) for s in scale)
    nc = _prog_cache.get(inv_scale)
    if nc is None:
        nc = _build_program(inv_scale)
        _prog_cache[inv_scale] = nc

    in_maps = _make_in_maps(
        {"x": x, "w_qkv": w_qkv, "w_out": w_out, "b_out": b_out})

    res = run_bass_kernel_spmd(nc, in_maps, core_ids=list(range(NCORES)))
    out = np.empty((B, N, D), dtype=np.float32)
    for c in range(NCORES):
        bi, qi = c // 4, c % 4
        out[bi, qi * NQ:(qi + 1) * NQ] = res.results[c]["y"]
    return out


# revision 19
# speedup vs baseline: 1.6033x; 1.0208x over previous
"""CosineSimilarityAttention Trainium2 kernel (8 NeuronCores, SPMD).

Sharding: token-parallel. Global tokens = 2 batches x 4096. Core c handles
batch (c // 4), query rows (c % 4)*1024 .. +1024. Each core computes K/V
projections for its whole batch (4096 tokens), plus Q for its own 1024
tokens, then attention and the output projection for its token slice.

Math per batch (faithful to reference):
  qkv = x @ w_qkv.T ; split q,k,v ; reshape heads h=12, dh=64
  q *= 1/sqrt(||q||_heads + eps)   (L2 norm over the HEADS axis, per (n, dh))
  k *= 1/sqrt(||k||_heads + eps)
  out_h = softmax((q_h k_h^T) / scale_h) v_h   (no max-subtract: |logits|<~2)
  y = concat_h(out_h) @ w_out.T + b_out

Perf notes (measured on this stack):
  - partial-array matmuls (K=64 row tiles / M<=64 col tiles) do NOT
    register as PE activity for the HAM clock gate and do NOT run
    concurrently here (walrus --enable-ldw-opt=false): phase A must use
    full-K zero-padded score matmuls + the 65-column attnV trick.
  - W+P is software-pipelined (transpose/drain of block b+1 is emitted
    before the norm chain of block b) so the PE never idles >3.4us and
    the clock gate stays at 8/8.
  - softmax normalization is per-head-pair: denominators staged to
    partitions {0,64} (DVE partition-base alignment), one reciprocal per
    pair, selector matmul broadcasts 1/den; runs under the next pair's
    exp wall instead of 24 flat-cost 3.3us reciprocals on the tail.
  - output projection contracts head PAIRS (K=128) from the pair-stacked
    attention-output layout.
"""

import numpy as np

import concourse.bass as bass
import concourse.mybir as mybir
import concourse.tile as tile
from concourse.bass_utils import run_bass_kernel_spmd
from concourse.masks import make_identity

F32 = mybir.dt.float32
BF16 = mybir.dt.bfloat16
AF = mybir.ActivationFunctionType

B = 2
N = 4096          # tokens per batch
D = 768           # model dim
H = 12            # heads
DH = 64           # head dim
INNER = H * DH    # 768
EPS = 1e-8
NQ = 1024         # query tokens per core
NCORES = 8
BLK = 512         # projection token block
KT = N // 128     # 32 key tiles of 128


def _split_multi_waits(nc):
    """This container's walrus accepts only ONE sync-wait per instruction.
    Hoist extra waits into standalone EVSEM instructions placed just before."""
    n = 0
    for f in nc.m.functions:
        for bb in f.blocks:
            insts = list(bb.instructions)
            out = []
            for inst in insts:
                si = inst.sync_info
                if si is not None and si.on_wait is not None and len(si.on_wait) > 1:
                    waits = list(si.on_wait)
                    for j, w in enumerate(waits[:-1]):
                        ev = mybir.InstEventSemaphore(
                            name=f"{inst.name}-evw{j}",
                            engine=inst.engine,
                            sync_info=mybir.SyncInfo(on_wait=[w], on_update=[]),
                        )
                        out.append(ev)
                        n += 1
                    si.on_wait = [waits[-1]]
                out.append(inst)
            bb.instructions = out
    return n


def _build_program(inv_scale):
    """Build the single SPMD Bass program. inv_scale: list of 12 floats."""
    nc = bass.Bass()
    xb = nc.declare_dram_parameter("xb", [N, D], F32, isOutput=False)
    qx = nc.declare_dram_parameter("qx", [NQ, D], F32, isOutput=False)
    wqkvT = nc.declare_dram_parameter("wqkvT", [D, 3 * INNER], F32, isOutput=False)
    woT = nc.declare_dram_parameter("woT", [INNER, D], F32, isOutput=False)
    bout = nc.declare_dram_parameter("bout", [1, D], F32, isOutput=False)
    selin = nc.declare_dram_parameter("selin", [128, 128], F32, isOutput=False)
    selqin = nc.declare_dram_parameter("selqin", [128, 128], F32, isOutput=False)
    y = nc.declare_dram_parameter("y", [NQ, D], F32, isOutput=True)

    with tile.TileContext(nc) as tc:
        with tc.tile_pool(name="const", bufs=1) as constp, \
             tc.tile_pool(name="persist", bufs=1) as persist:
            # --- constants ---
            ident = constp.tile([128, 128], F32)
            make_identity(nc, ident)
            sel_bf = constp.tile([128, 128], BF16)
            selq_bf = constp.tile([128, 128], BF16)
            ones_f = constp.tile([1, 64], F32)
            nc.vector.memset(ones_f, 1.0)
            ones_bf = constp.tile([1, 128], BF16)
            nc.vector.memset(ones_bf, 1.0)
            eps_t = constp.tile([128, 1], F32)
            nc.vector.memset(eps_t, EPS)
            invs = constp.tile([128, 6], F32)
            for dt in range(6):
                nc.vector.memset(invs[0:64, dt:dt + 1], float(inv_scale[2 * dt]))
                nc.vector.memset(invs[64:128, dt:dt + 1],
                                 float(inv_scale[2 * dt + 1]))
            b_bf = constp.tile([1, D], BF16)
            with tc.tile_pool(name="cstage", bufs=1) as cst:
                sel_st = cst.tile([128, 128], F32, tag="sel")
                nc.sync.dma_start(out=sel_st, in_=selin[:, :])
                nc.vector.tensor_copy(sel_bf, sel_st)

            # --- persistent activations ---
            khat = persist.tile([128, 6, N], BF16)     # k^T normed [dim, tok]
            qhat = persist.tile([128, H, NQ], BF16)    # q^T per head, zero-pad
            vhat = persist.tile([128, KT, H * 65], BF16)  # v [tok, h*65] (+ones)
            o2_all = persist.tile([128, 6, NQ], BF16)  # attn out, pair-stacked
            wo12 = persist.tile([128, 6, D], BF16)     # w_out pair-stacked

            # ---------------- phase W+P: weights, projections, head-norm ----
            with tc.tile_pool(name="pw", bufs=1) as pwp:
              wq = pwp.tile([128, 6, 3 * INNER], BF16)
              with tc.tile_pool(name="wstage", bufs=1) as wst:
                for dt in range(6):
                    st = wst.tile([128, 3 * INNER], F32, tag="wst")
                    nc.scalar.dma_start(out=st,
                                        in_=wqkvT[dt * 128:(dt + 1) * 128, :])
                    nc.vector.tensor_copy(wq[:, dt, :], st)
              with tc.tile_pool(name="pstage", bufs=2) as pstage, \
                   tc.tile_pool(name="wostage", bufs=1) as wost, \
                   tc.tile_pool(name="pxT", bufs=2) as pxT, \
                   tc.tile_pool(name="pkf", bufs=1) as pkf, \
                   tc.tile_pool(name="pksq", bufs=1) as pksq, \
                   tc.tile_pool(name="psmall", bufs=1) as psmall, \
                   tc.tile_pool(name="psumTP", bufs=1, space="PSUM") as pTP, \
                   tc.tile_pool(name="psumKP", bufs=2, space="PSUM") as pKP, \
                   tc.tile_pool(name="psumSQ", bufs=1, space="PSUM") as pSQ, \
                   tc.tile_pool(name="psumVP", bufs=2, space="PSUM") as pVP:

                  def load_transpose(src, blk_i):
                      # DMA x block [512, D] + transpose -> xT [dim, tok] bf16.
                      # Emitted one block ahead of the compute part so the
                      # PE's projection matmuls never wait on the DVE drains.
                      xsts = []
                      for hh in range(2):
                          xst = pstage.tile([128, 2, D], F32, tag="xst")
                          nc.sync.dma_start(
                              out=xst,
                              in_=src[blk_i * BLK + hh * 256:
                                      blk_i * BLK + (hh + 1) * 256, :].rearrange(
                                  "(t p) d -> p t d", p=128),
                          )
                          xsts.append(xst)
                      xT = pxT.tile([128, 6, BLK], BF16, tag="xT")
                      for dt in range(6):
                          tp = pTP.tile([128, 512], F32, tag="pTP")
                          for tt in range(4):
                              nc.tensor.transpose(
                                  tp[:, tt * 128:(tt + 1) * 128],
                                  xsts[tt // 2][:, tt % 2,
                                                dt * 128:(dt + 1) * 128], ident)
                          nc.vector.tensor_copy(xT[:, dt, :], tp)
                      return xT

                  def proj_compute(xT, blk_i, is_q):
                      wbase = 0 if is_q else INNER
                      # q^T / k^T projection [dim_out, tok]; kf bf16 via
                      # ScalarE, squares straight from PSUM via ScalarE Square
                      kf = pkf.tile([128, 6, BLK], BF16, tag="kf")
                      sq = pSQ.tile([128, 512], F32, tag="pSQ")
                      for dt in range(6):
                          kp = pKP.tile([128, 512], F32, tag="pKP")
                          for ks in range(6):
                              nc.tensor.matmul(
                                  kp,
                                  wq[:, ks, wbase + dt * 128: wbase + (dt + 1) * 128],
                                  xT[:, ks, :],
                                  start=(ks == 0), stop=(ks == 5))
                          nc.scalar.copy(kf[:, dt, :], kp)
                          ksq = pksq.tile([128, BLK], BF16, tag="ksq")
                          nc.scalar.square(ksq, kp)
                          nc.tensor.matmul(sq, sel_bf, ksq,
                                           start=(dt == 0), stop=(dt == 5))
                      nrm = psmall.tile([128, BLK], F32, tag="nrm")
                      nc.scalar.activation(nrm, sq, AF.Sqrt)
                      u = pSQ.tile([128, BLK], F32, tag="pSQ")
                      nc.scalar.activation(u, nrm, AF.Sqrt, bias=eps_t[:, :])
                      rq = psmall.tile([128, BLK], F32, tag="nrm")
                      nc.vector.reciprocal(rq, u)
                      bsl = bass.ts(blk_i, BLK)
                      if is_q:
                          # zero-padded per-head layout: head 2dt on rows 0:64,
                          # head 2dt+1 on rows 64:128, other rows stay zero.
                          # 1/scale_h is folded in so exp needs no scale.
                          for dt in range(6):
                              a = qhat[0:64, 2 * dt, bsl]
                              b = qhat[64:128, 2 * dt + 1, bsl]
                              nc.vector.tensor_mul(a, kf[0:64, dt, :], rq[0:64, :])
                              nc.vector.tensor_mul(b, kf[64:128, dt, :],
                                                   rq[64:128, :])
                              nc.vector.tensor_scalar_mul(a, a,
                                                          invs[0:64, dt:dt + 1])
                              nc.vector.tensor_scalar_mul(b, b,
                                                          invs[64:128, dt:dt + 1])
                          return
                      for dt in range(6):
                          nc.vector.tensor_mul(
                              khat[:, dt, bsl], kf[:, dt, :], rq)
                      # v projection [tok, inner] -> vhat strided 65
                      for tt in range(4):
                          vp = pVP.tile([128, 1024], F32, tag="pVP")
                          for ks in range(6):
                              nc.tensor.matmul(vp[:, 0:512],
                                               xT[:, ks, tt * 128:(tt + 1) * 128],
                                               wq[:, ks, 2 * INNER:2 * INNER + 512],
                                               start=(ks == 0), stop=(ks == 5))
                              nc.tensor.matmul(vp[:, 512:768],
                                               xT[:, ks, tt * 128:(tt + 1) * 128],
                                               wq[:, ks, 2 * INNER + 512:3 * INNER],
                                               start=(ks == 0), stop=(ks == 5))
                          vdst = vhat[:, blk_i * 4 + tt, :].rearrange(
                              "p (h c) -> p h c", c=65)[:, :, 0:64]
                          nc.vector.tensor_copy(
                              vdst, vp[:, 0:768].rearrange("p (h c) -> p h c", c=64))

                  # software pipeline: transpose part of block i+1 lands in
                  # the engine queues before the norm chain of block i
                  def stage_wo(h):
                      wst_t = wost.tile([64, D], F32, tag="wost")
                      nc.scalar.dma_start(out=wst_t,
                                          in_=woT[h * 64:(h + 1) * 64, :])
                      base = (h % 2) * 64
                      nc.vector.tensor_copy(wo12[base:base + 64, h // 2, :],
                                            wst_t)

                  staging = [lambda h=h: stage_wo(h) for h in range(H)]

                  blocks = [(qx, b, True) for b in range(NQ // BLK)] + \
                           [(xb, b, False) for b in range(N // BLK)]
                  xT_pend = load_transpose(blocks[0][0], blocks[0][1])
                  # memsets after the first block's DMA+drains are queued
                  vones = vhat.rearrange(
                      "p t (h c) -> p t h c", c=65)[:, :, :, 64:65]
                  nc.vector.memset(vones, 1.0)
                  nc.vector.memset(qhat, 0.0)
                  for i, (src, b, is_q) in enumerate(blocks):
                      xT_cur = xT_pend
                      if i + 1 < len(blocks):
                          nsrc, nb, _ = blocks[i + 1]
                          xT_pend = load_transpose(nsrc, nb)
                      proj_compute(xT_cur, b, is_q)
                      if i >= 2:
                          for _ in range(2):
                              if staging:
                                  staging.pop(0)()

            # ---------------- phase A: attention ----------------
            with tc.tile_pool(name="pP", bufs=6) as pP, \
                 tc.tile_pool(name="pden", bufs=2) as pden, \
                 tc.tile_pool(name="princ", bufs=2) as princ, \
                 tc.tile_pool(name="psumS", bufs=2, space="PSUM") as pS, \
                 tc.tile_pool(name="psumO", bufs=4, space="PSUM") as pO:
                # head-pair processing: heads (2hp, 2hp+1) live on PE row
                # groups 0-63 / 64-127 via the zero-padded qhat. Queries in
                # 512-halves so every PSUM tile is one bank.
                for nm, dst, srcdram, shp in (
                        ("selq", selq_bf, selqin, [128, 128]),
                        ("bias", b_bf, bout, [1, D])):
                    stg = pden.tile(shp, F32, tag="den", name=f"stg_{nm}")
                    nc.scalar.dma_start(out=stg, in_=srcdram[:, :])
                    nc.vector.tensor_copy(dst, stg)

                def norm_pair(hp, ots):
                    # drain + normalize pair hp: denominators staged to
                    # partitions {0, 64}, one reciprocal, selector matmul
                    # broadcasts 1/den. Deferred into the NEXT pair's kb
                    # stream so the PE queue never idles on the DVE chain.
                    den_st = pden.tile([128, 1024], F32, tag="den",
                                       name=f"den_{hp}")
                    nc.vector.memset(den_st, 1.0)
                    for j in range(2):
                        for qh in range(2):
                            qsl = bass.ts(qh, 512)
                            nc.vector.tensor_copy(
                                o2_all[64 * j:64 * j + 64, hp, qsl],
                                ots[(j, qh)][0:64, :])
                            nc.vector.tensor_copy(
                                den_st[64 * j:64 * j + 1, qsl],
                                ots[(j, qh)][64:65, :])
                    rinv = princ.tile([128, 1024], BF16, tag="rinv",
                                      name=f"rinv_{hp}")
                    with nc.allow_low_precision(
                            reason="bf16 1/den: 0.4% rel on softmax scale, "
                                   "well under the 2e-2 budget"):
                        nc.vector.reciprocal(rinv, den_st)
                    for qh in range(2):
                        qsl = bass.ts(qh, 512)
                        rbc = pS.tile([128, 512], F32, tag="pS",
                                      name=f"rbc_{qh}")
                        nc.tensor.matmul(rbc, selq_bf, rinv[:, qsl],
                                         start=True, stop=True)
                        nc.vector.tensor_mul(
                            o2_all[:, hp, qsl], o2_all[:, hp, qsl], rbc)

                pending = None
                for hp in range(6):
                    ots = {}
                    for j in range(2):
                        for qh in range(2):
                            ots[(j, qh)] = pO.tile([65, 512], F32, tag="pO",
                                                   name=f"ot_{hp}_{j}_{qh}")
                    for kb in range(KT):
                        kbsl = bass.ts(kb, 128)
                        sts = {}
                        for qh in range(2):
                            qsl = bass.ts(qh, 512)
                            st = pS.tile([128, 1024], F32, tag="pS",
                                         name=f"st_{qh}")
                            for j in range(2):
                                nc.tensor.matmul(
                                    st[:, j * 512:(j + 1) * 512],
                                    khat[:, hp, kbsl],
                                    qhat[:, 2 * hp + j, qsl],
                                    start=True, stop=True)
                            sts[qh] = st
                        for qh in range(2):
                            pt = pP.tile([128, 1024], BF16, tag="pP",
                                         name=f"pt_{qh}")
                            nc.scalar.activation(pt, sts[qh], AF.Exp)
                            for j in range(2):
                                h = 2 * hp + j
                                nc.tensor.matmul(
                                    ots[(j, qh)],
                                    vhat[:, kb, h * 65:(h + 1) * 65],
                                    pt[:, j * 512:(j + 1) * 512],
                                    start=(kb == 0), stop=(kb == KT - 1))
                        if kb == 6 and pending is not None:
                            pending()
                            pending = None
                    pending = (lambda hp=hp, ots=ots: norm_pair(hp, ots))
                pending()

            # ---------------- phase Y: output projection ----------------
            with tc.tile_pool(name="pys", bufs=2) as pys, \
                 tc.tile_pool(name="psumY", bufs=2, space="PSUM") as pY:
                for mt in range(NQ // 128):
                    mtsl = bass.ts(mt, 128)
                    yp = pY.tile([128, 1024], F32, tag="pY")
                    for hp in range(6):
                        lhsT = o2_all[:, hp, mtsl]
                        nc.tensor.matmul(yp[:, 0:512], lhsT, wo12[:, hp, 0:512],
                                         start=(hp == 0), stop=False)
                        nc.tensor.matmul(yp[:, 512:768], lhsT, wo12[:, hp, 512:768],
                                         start=(hp == 0), stop=False)
                    nc.tensor.matmul(yp[:, 0:512], ones_bf, b_bf[:, 0:512],
                                     start=False, stop=True)
                    nc.tensor.matmul(yp[:, 512:768], ones_bf, b_bf[:, 512:768],
                                     start=False, stop=True)
                    ys = pys.tile([128, D], F32, tag="ys")
                    nc.vector.tensor_copy(ys, yp[:, 0:768])
                    nc.sync.dma_start(out=y[mt * 128:(mt + 1) * 128, :], in_=ys)

    _split_multi_waits(nc)
    return nc


_prog_cache = {}


def _make_in_maps(inputs):
    x = np.ascontiguousarray(np.asarray(inputs["x"], dtype=np.float32))
    w_qkv = np.asarray(inputs["w_qkv"], dtype=np.float32)
    w_out = np.asarray(inputs["w_out"], dtype=np.float32)
    b_out = np.asarray(inputs["b_out"], dtype=np.float32).reshape(1, D)

    wqkvT = np.ascontiguousarray(w_qkv.T)            # [768, 2304]
    woT = np.ascontiguousarray(w_out.T)              # [768, 768]
    p = np.arange(128)
    sel = (p[:, None] % 64 == p[None, :] % 64).astype(np.float32)
    # selq[r, c] = 1 where r == 64*(c >= 64): broadcasts the 1/den rows
    # (staged at partitions 0/64) to the pair-stacked [128, q] layout
    selq = (p[:, None] == 64 * (p[None, :] >= 64)).astype(np.float32)

    in_maps = []
    for cix in range(NCORES):
        bi, qi = cix // 4, cix % 4
        in_maps.append({
            "xb": x[bi],
            "qx": np.ascontiguousarray(x[bi, qi * NQ:(qi + 1) * NQ]),
            "wqkvT": wqkvT,
            "woT": woT,
            "bout": b_out,
            "selin": sel,
            "selqin": selq,
        })
    return in_maps


def kernel(x, w_qkv, w_out, b_out, scale):
    scale = np.asarray(scale, dtype=np.float32)
    inv_scale = tuple(float(1.0 / s) for s in scale)
    nc = _prog_cache.get(inv_scale)
    if nc is None:
        nc = _build_program(inv_scale)
        _prog_cache[inv_scale] = nc

    in_maps = _make_in_maps(
        {"x": x, "w_qkv": w_qkv, "w_out": w_out, "b_out": b_out})

    res = run_bass_kernel_spmd(nc, in_maps, core_ids=list(range(NCORES)))
    out = np.empty((B, N, D), dtype=np.float32)
    for c in range(NCORES):
        bi, qi = c // 4, c % 4
        out[bi, qi * NQ:(qi + 1) * NQ] = res.results[c]["y"]
    return out


# revision 23
# speedup vs baseline: 1.6365x; 1.0207x over previous
"""CosineSimilarityAttention Trainium2 kernel (8 NeuronCores, SPMD).

Sharding: token-parallel. Global tokens = 2 batches x 4096. Core c handles
batch (c // 4), query rows (c % 4)*1024 .. +1024. Each core computes K/V
projections for its whole batch (4096 tokens), plus Q for its own 1024
tokens, then attention and the output projection for its token slice.

Math per batch (faithful to reference):
  qkv = x @ w_qkv.T ; split q,k,v ; reshape heads h=12, dh=64
  q *= 1/sqrt(||q||_heads + eps)   (L2 norm over the HEADS axis, per (n, dh))
  k *= 1/sqrt(||k||_heads + eps)
  out_h = softmax((q_h k_h^T) / scale_h) v_h   (no max-subtract: |logits|<~2)
  y = concat_h(out_h) @ w_out.T + b_out

Perf notes (measured on this stack):
  - partial-array matmuls (K=64 row tiles / M<=64 col tiles) do NOT
    register as PE activity for the HAM clock gate and do NOT run
    concurrently here (walrus --enable-ldw-opt=false): phase A must use
    full-K zero-padded score matmuls + the 65-column attnV trick.
  - W+P is software-pipelined (transpose/drain of block b+1 is emitted
    before the norm chain of block b) so the PE never idles >3.4us and
    the clock gate stays at 8/8.
  - softmax normalization is per-head-pair: denominators staged to
    partitions {0,64} (DVE partition-base alignment), one reciprocal per
    pair, selector matmul broadcasts 1/den; runs under the next pair's
    exp wall instead of 24 flat-cost 3.3us reciprocals on the tail.
  - output projection contracts head PAIRS (K=128) from the pair-stacked
    attention-output layout.
"""

import numpy as np

import concourse.bass as bass
import concourse.mybir as mybir
import concourse.tile as tile
from concourse.bass_utils import run_bass_kernel_spmd
from concourse.masks import make_identity

F32 = mybir.dt.float32
BF16 = mybir.dt.bfloat16
AF = mybir.ActivationFunctionType

B = 2
N = 4096          # tokens per batch
D = 768           # model dim
H = 12            # heads
DH = 64           # head dim
INNER = H * DH    # 768
EPS = 1e-8
NQ = 1024         # query tokens per core
NCORES = 8
BLK = 512         # projection token block
KT = N // 128     # 32 key tiles of 128


def _split_multi_waits(nc):
    """This container's walrus accepts only ONE sync-wait per instruction.
    Hoist extra waits into standalone EVSEM instructions placed just before."""
    n = 0
    for f in nc.m.functions:
        for bb in f.blocks:
            insts = list(bb.instructions)
            out = []
            for inst in insts:
                si = inst.sync_info
                if si is not None and si.on_wait is not None and len(si.on_wait) > 1:
                    waits = list(si.on_wait)
                    for j, w in enumerate(waits[:-1]):
                        ev = mybir.InstEventSemaphore(
                            name=f"{inst.name}-evw{j}",
                            engine=inst.engine,
                            sync_info=mybir.SyncInfo(on_wait=[w], on_update=[]),
                        )
                        out.append(ev)
                        n += 1
                    si.on_wait = [waits[-1]]
                out.append(inst)
            bb.instructions = out
    return n


def _build_program(inv_scale):
    """Build the single SPMD Bass program. inv_scale: list of 12 floats."""
    nc = bass.Bass()
    xb = nc.declare_dram_parameter("xb", [N, D], F32, isOutput=False)
    qx = nc.declare_dram_parameter("qx", [NQ, D], F32, isOutput=False)
    wqkvT = nc.declare_dram_parameter("wqkvT", [D, 3 * INNER], F32, isOutput=False)
    woT = nc.declare_dram_parameter("woT", [INNER, D], F32, isOutput=False)
    bout = nc.declare_dram_parameter("bout", [1, D], F32, isOutput=False)
    selin = nc.declare_dram_parameter("selin", [128, 128], F32, isOutput=False)
    selqin = nc.declare_dram_parameter("selqin", [128, 128], F32, isOutput=False)
    y = nc.declare_dram_parameter("y", [NQ, D], F32, isOutput=True)

    with tile.TileContext(nc) as tc:
        with tc.tile_pool(name="const", bufs=1) as constp, \
             tc.tile_pool(name="persist", bufs=1) as persist:
            # --- constants ---
            ident = constp.tile([128, 128], F32)
            make_identity(nc, ident)
            sel_bf = constp.tile([128, 128], BF16)
            selq_bf = constp.tile([128, 128], BF16)
            ones_f = constp.tile([1, 64], F32)
            nc.vector.memset(ones_f, 1.0)
            ones_bf = constp.tile([1, 128], BF16)
            nc.vector.memset(ones_bf, 1.0)
            eps_t = constp.tile([128, 1], F32)
            nc.vector.memset(eps_t, EPS)
            invs = constp.tile([128, 6], F32)
            for dt in range(6):
                nc.vector.memset(invs[0:64, dt:dt + 1], float(inv_scale[2 * dt]))
                nc.vector.memset(invs[64:128, dt:dt + 1],
                                 float(inv_scale[2 * dt + 1]))
            b_bf = constp.tile([1, D], BF16)

            # --- persistent activations ---
            khat = persist.tile([128, 6, N], BF16)     # k^T normed [dim, tok]
            qhat = persist.tile([128, H, NQ], BF16)    # q^T per head, zero-pad
            vhat = persist.tile([128, KT, H * 65], BF16)  # v [tok, h*65] (+ones)
            o2_all = persist.tile([128, 6, NQ], BF16)  # attn out, pair-stacked
            wo12 = persist.tile([128, 6, D], BF16)     # w_out pair-stacked

            # ---------------- phase W+P: weights, projections, head-norm ----
            with tc.tile_pool(name="pw", bufs=1) as pwp:
              wq = pwp.tile([128, 6, 3 * INNER], BF16)
              with tc.tile_pool(name="wstage", bufs=2) as wst, \
                   tc.tile_pool(name="cstage", bufs=1) as cst:
                # qkv-weight DMAs go first on the DVE-triggered queue so the
                # x-block loads (sync queue) run in parallel; casts on the
                # otherwise-idle ScalarE keep the DVE head clear for the
                # first transpose drains.
                for dt in range(6):
                    st = wst.tile([128, 3 * INNER], F32, tag="wst",
                                  name=f"wst_{dt}")
                    nc.gpsimd.dma_start(
                        out=st, in_=wqkvT[dt * 128:(dt + 1) * 128, :])
                    nc.scalar.copy(wq[:, dt, :], st)
                sel_st = cst.tile([128, 128], F32, tag="sel")
                nc.sync.dma_start(out=sel_st, in_=selin[:, :])
                nc.vector.tensor_copy(sel_bf, sel_st)
              with tc.tile_pool(name="pstage", bufs=2) as pstage, \
                   tc.tile_pool(name="wostage", bufs=1) as wost, \
                   tc.tile_pool(name="pxT", bufs=2) as pxT, \
                   tc.tile_pool(name="pkf", bufs=1) as pkf, \
                   tc.tile_pool(name="pksq", bufs=1) as pksq, \
                   tc.tile_pool(name="psmall", bufs=1) as psmall, \
                   tc.tile_pool(name="psumTP", bufs=1, space="PSUM") as pTP, \
                   tc.tile_pool(name="psumKP", bufs=2, space="PSUM") as pKP, \
                   tc.tile_pool(name="psumSQ", bufs=1, space="PSUM") as pSQ, \
                   tc.tile_pool(name="psumVP", bufs=2, space="PSUM") as pVP:

                  def load_transpose(src, blk_i):
                      # DMA x block [512, D] + transpose -> xT [dim, tok] bf16.
                      # Emitted one block ahead of the compute part so the
                      # PE's projection matmuls never wait on the DVE drains.
                      xsts = []
                      for hh in range(2):
                          xst = pstage.tile([128, 2, D], F32, tag="xst")
                          nc.sync.dma_start(
                              out=xst,
                              in_=src[blk_i * BLK + hh * 256:
                                      blk_i * BLK + (hh + 1) * 256, :].rearrange(
                                  "(t p) d -> p t d", p=128),
                          )
                          xsts.append(xst)
                      xT = pxT.tile([128, 6, BLK], BF16, tag="xT")
                      for dt in range(6):
                          tp = pTP.tile([128, 512], F32, tag="pTP")
                          for tt in range(4):
                              nc.tensor.transpose(
                                  tp[:, tt * 128:(tt + 1) * 128],
                                  xsts[tt // 2][:, tt % 2,
                                                dt * 128:(dt + 1) * 128], ident)
                          nc.vector.tensor_copy(xT[:, dt, :], tp)
                      return xT

                  def proj_compute(xT, blk_i, is_q):
                      wbase = 0 if is_q else INNER
                      # q^T / k^T projection [dim_out, tok]; kf bf16 via
                      # ScalarE, squares straight from PSUM via ScalarE Square
                      kf = pkf.tile([128, 6, BLK], BF16, tag="kf")
                      sq = pSQ.tile([128, 512], F32, tag="pSQ")
                      for dt in range(6):
                          kp = pKP.tile([128, 512], F32, tag="pKP")
                          for ks in range(6):
                              nc.tensor.matmul(
                                  kp,
                                  wq[:, ks, wbase + dt * 128: wbase + (dt + 1) * 128],
                                  xT[:, ks, :],
                                  start=(ks == 0), stop=(ks == 5))
                          nc.scalar.copy(kf[:, dt, :], kp)
                          ksq = pksq.tile([128, BLK], BF16, tag="ksq")
                          nc.scalar.square(ksq, kp)
                          nc.tensor.matmul(sq, sel_bf, ksq,
                                           start=(dt == 0), stop=(dt == 5))
                      nrm = psmall.tile([128, BLK], F32, tag="nrm")
                      nc.scalar.activation(nrm, sq, AF.Sqrt)
                      u = pSQ.tile([128, BLK], F32, tag="pSQ")
                      nc.scalar.activation(u, nrm, AF.Sqrt, bias=eps_t[:, :])
                      rq = psmall.tile([128, BLK], F32, tag="nrm")
                      nc.vector.reciprocal(rq, u)
                      bsl = bass.ts(blk_i, BLK)
                      if is_q:
                          # zero-padded per-head layout: head 2dt on rows 0:64,
                          # head 2dt+1 on rows 64:128, other rows stay zero.
                          # 1/scale_h is folded in so exp needs no scale.
                          for dt in range(6):
                              a = qhat[0:64, 2 * dt, bsl]
                              b = qhat[64:128, 2 * dt + 1, bsl]
                              nc.vector.tensor_mul(a, kf[0:64, dt, :], rq[0:64, :])
                              nc.vector.tensor_mul(b, kf[64:128, dt, :],
                                                   rq[64:128, :])
                              nc.vector.tensor_scalar_mul(a, a,
                                                          invs[0:64, dt:dt + 1])
                              nc.vector.tensor_scalar_mul(b, b,
                                                          invs[64:128, dt:dt + 1])
                          return
                      for dt in range(6):
                          nc.vector.tensor_mul(
                              khat[:, dt, bsl], kf[:, dt, :], rq)
                      # v projection [tok, inner] -> vhat strided 65
                      for tt in range(4):
                          vp = pVP.tile([128, 1024], F32, tag="pVP")
                          for ks in range(6):
                              nc.tensor.matmul(vp[:, 0:512],
                                               xT[:, ks, tt * 128:(tt + 1) * 128],
                                               wq[:, ks, 2 * INNER:2 * INNER + 512],
                                               start=(ks == 0), stop=(ks == 5))
                              nc.tensor.matmul(vp[:, 512:768],
                                               xT[:, ks, tt * 128:(tt + 1) * 128],
                                               wq[:, ks, 2 * INNER + 512:3 * INNER],
                                               start=(ks == 0), stop=(ks == 5))
                          vdst = vhat[:, blk_i * 4 + tt, :].rearrange(
                              "p (h c) -> p h c", c=65)[:, :, 0:64]
                          nc.vector.tensor_copy(
                              vdst, vp[:, 0:768].rearrange("p (h c) -> p h c", c=64))

                  # software pipeline: transpose part of block i+1 lands in
                  # the engine queues before the norm chain of block i
                  def stage_wo(h):
                      wst_t = wost.tile([64, D], F32, tag="wost")
                      nc.scalar.dma_start(out=wst_t,
                                          in_=woT[h * 64:(h + 1) * 64, :])
                      base = (h % 2) * 64
                      nc.vector.tensor_copy(wo12[base:base + 64, h // 2, :],
                                            wst_t)

                  staging = [lambda h=h: stage_wo(h) for h in range(H)]

                  blocks = [(qx, b, True) for b in range(NQ // BLK)] + \
                           [(xb, b, False) for b in range(N // BLK)]
                  xT_pend = load_transpose(blocks[0][0], blocks[0][1])
                  vones = vhat.rearrange(
                      "p t (h c) -> p t h c", c=65)[:, :, :, 64:65]
                  nc.vector.memset(vones, 1.0)
                  nc.vector.memset(qhat, 0.0)
                  for i, (src, b, is_q) in enumerate(blocks):
                      xT_cur = xT_pend
                      if i + 1 < len(blocks):
                          nsrc, nb, _ = blocks[i + 1]
                          xT_pend = load_transpose(nsrc, nb)
                      proj_compute(xT_cur, b, is_q)
                      if i >= 2:
                          for _ in range(2):
                              if staging:
                                  staging.pop(0)()

            # ---------------- phase A: attention ----------------
            with tc.tile_pool(name="pP", bufs=6) as pP, \
                 tc.tile_pool(name="pden", bufs=2) as pden, \
                 tc.tile_pool(name="princ", bufs=2) as princ, \
                 tc.tile_pool(name="psumS", bufs=2, space="PSUM") as pS, \
                 tc.tile_pool(name="psumO", bufs=4, space="PSUM") as pO:
                # head-pair processing: heads (2hp, 2hp+1) live on PE row
                # groups 0-63 / 64-127 via the zero-padded qhat. Queries in
                # 512-halves so every PSUM tile is one bank.
                for nm, dst, srcdram, shp in (
                        ("selq", selq_bf, selqin, [128, 128]),
                        ("bias", b_bf, bout, [1, D])):
                    stg = pden.tile(shp, F32, tag="den", name=f"stg_{nm}")
                    nc.scalar.dma_start(out=stg, in_=srcdram[:, :])
                    nc.vector.tensor_copy(dst, stg)

                def norm_pair(hp, ots):
                    # drain + normalize pair hp: denominators staged to
                    # partitions {0, 64}, one reciprocal, selector matmul
                    # broadcasts 1/den. Deferred into the NEXT pair's kb
                    # stream so the PE queue never idles on the DVE chain.
                    den_st = pden.tile([128, 1024], F32, tag="den",
                                       name=f"den_{hp}")
                    nc.vector.memset(den_st, 1.0)
                    for j in range(2):
                        for qh in range(2):
                            qsl = bass.ts(qh, 512)
                            nc.vector.tensor_copy(
                                o2_all[64 * j:64 * j + 64, hp, qsl],
                                ots[(j, qh)][0:64, :])
                            nc.vector.tensor_copy(
                                den_st[64 * j:64 * j + 1, qsl],
                                ots[(j, qh)][64:65, :])
                    rinv = princ.tile([128, 1024], BF16, tag="rinv",
                                      name=f"rinv_{hp}")
                    with nc.allow_low_precision(
                            reason="bf16 1/den: 0.4% rel on softmax scale, "
                                   "well under the 2e-2 budget"):
                        nc.vector.reciprocal(rinv, den_st)
                    for qh in range(2):
                        qsl = bass.ts(qh, 512)
                        rbc = pS.tile([128, 512], F32, tag="pS",
                                      name=f"rbc_{qh}")
                        nc.tensor.matmul(rbc, selq_bf, rinv[:, qsl],
                                         start=True, stop=True)
                        nc.vector.tensor_mul(
                            o2_all[:, hp, qsl], o2_all[:, hp, qsl], rbc)

                pending = None
                for hp in range(6):
                    ots = {}
                    for j in range(2):
                        for qh in range(2):
                            ots[(j, qh)] = pO.tile([65, 512], F32, tag="pO",
                                                   name=f"ot_{hp}_{j}_{qh}")
                    for kb in range(KT):
                        kbsl = bass.ts(kb, 128)
                        sts = {}
                        for qh in range(2):
                            qsl = bass.ts(qh, 512)
                            st = pS.tile([128, 1024], F32, tag="pS",
                                         name=f"st_{qh}")
                            for j in range(2):
                                nc.tensor.matmul(
                                    st[:, j * 512:(j + 1) * 512],
                                    khat[:, hp, kbsl],
                                    qhat[:, 2 * hp + j, qsl],
                                    start=True, stop=True)
                            sts[qh] = st
                        for qh in range(2):
                            pt = pP.tile([128, 1024], BF16, tag="pP",
                                         name=f"pt_{qh}")
                            nc.scalar.activation(pt, sts[qh], AF.Exp)
                            for j in range(2):
                                h = 2 * hp + j
                                nc.tensor.matmul(
                                    ots[(j, qh)],
                                    vhat[:, kb, h * 65:(h + 1) * 65],
                                    pt[:, j * 512:(j + 1) * 512],
                                    start=(kb == 0), stop=(kb == KT - 1))
                        if kb == 6 and pending is not None:
                            pending()
                            pending = None
                    pending = (lambda hp=hp, ots=ots: norm_pair(hp, ots))
                pending()

            # ---------------- phase Y: output projection ----------------
            with tc.tile_pool(name="pys", bufs=2) as pys, \
                 tc.tile_pool(name="psumY", bufs=2, space="PSUM") as pY:
                for mt in range(NQ // 128):
                    mtsl = bass.ts(mt, 128)
                    yp = pY.tile([128, 1024], F32, tag="pY")
                    for hp in range(6):
                        lhsT = o2_all[:, hp, mtsl]
                        nc.tensor.matmul(yp[:, 0:512], lhsT, wo12[:, hp, 0:512],
                                         start=(hp == 0), stop=False)
                        nc.tensor.matmul(yp[:, 512:768], lhsT, wo12[:, hp, 512:768],
                                         start=(hp == 0), stop=False)
                    nc.tensor.matmul(yp[:, 0:512], ones_bf, b_bf[:, 0:512],
                                     start=False, stop=True)
                    nc.tensor.matmul(yp[:, 512:768], ones_bf, b_bf[:, 512:768],
                                     start=False, stop=True)
                    ys = pys.tile([128, D], F32, tag="ys")
                    nc.vector.tensor_copy(ys, yp[:, 0:768])
                    nc.sync.dma_start(out=y[mt * 128:(mt + 1) * 128, :], in_=ys)

    _split_multi_waits(nc)
    return nc


_prog_cache = {}


def _make_in_maps(inputs):
    x = np.ascontiguousarray(np.asarray(inputs["x"], dtype=np.float32))
    w_qkv = np.asarray(inputs["w_qkv"], dtype=np.float32)
    w_out = np.asarray(inputs["w_out"], dtype=np.float32)
    b_out = np.asarray(inputs["b_out"], dtype=np.float32).reshape(1, D)

    wqkvT = np.ascontiguousarray(w_qkv.T)            # [768, 2304]
    woT = np.ascontiguousarray(w_out.T)              # [768, 768]
    p = np.arange(128)
    sel = (p[:, None] % 64 == p[None, :] % 64).astype(np.float32)
    # selq[r, c] = 1 where r == 64*(c >= 64): broadcasts the 1/den rows
    # (staged at partitions 0/64) to the pair-stacked [128, q] layout
    selq = (p[:, None] == 64 * (p[None, :] >= 64)).astype(np.float32)

    in_maps = []
    for cix in range(NCORES):
        bi, qi = cix // 4, cix % 4
        in_maps.append({
            "xb": x[bi],
            "qx": np.ascontiguousarray(x[bi, qi * NQ:(qi + 1) * NQ]),
            "wqkvT": wqkvT,
            "woT": woT,
            "bout": b_out,
            "selin": sel,
            "selqin": selq,
        })
    return in_maps


def kernel(x, w_qkv, w_out, b_out, scale):
    scale = np.asarray(scale, dtype=np.float32)
    inv_scale = tuple(float(1.0 / s) for s in scale)
    nc = _prog_cache.get(inv_scale)
    if nc is None:
        nc = _build_program(inv_scale)
        _prog_cache[inv_scale] = nc

    in_maps = _make_in_maps(
        {"x": x, "w_qkv": w_qkv, "w_out": w_out, "b_out": b_out})

    res = run_bass_kernel_spmd(nc, in_maps, core_ids=list(range(NCORES)))
    out = np.empty((B, N, D), dtype=np.float32)
    for c in range(NCORES):
        bi, qi = c // 4, c % 4
        out[bi, qi * NQ:(qi + 1) * NQ] = res.results[c]["y"]
    return out


# revision 24
# speedup vs baseline: 1.6457x; 1.0056x over previous
"""CosineSimilarityAttention Trainium2 kernel (8 NeuronCores, SPMD).

Sharding: token-parallel. Global tokens = 2 batches x 4096. Core c handles
batch (c // 4), query rows (c % 4)*1024 .. +1024. Each core computes K/V
projections for its whole batch (4096 tokens), plus Q for its own 1024
tokens, then attention and the output projection for its token slice.

Math per batch (faithful to reference):
  qkv = x @ w_qkv.T ; split q,k,v ; reshape heads h=12, dh=64
  q *= 1/sqrt(||q||_heads + eps)   (L2 norm over the HEADS axis, per (n, dh))
  k *= 1/sqrt(||k||_heads + eps)
  out_h = softmax((q_h k_h^T) / scale_h) v_h   (no max-subtract: |logits|<~2)
  y = concat_h(out_h) @ w_out.T + b_out

Perf notes (measured on this stack):
  - partial-array matmuls (K=64 row tiles / M<=64 col tiles) do NOT
    register as PE activity for the HAM clock gate and do NOT run
    concurrently here (walrus --enable-ldw-opt=false): phase A must use
    full-K zero-padded score matmuls + the 65-column attnV trick.
  - W+P is software-pipelined (transpose/drain of block b+1 is emitted
    before the norm chain of block b) so the PE never idles >3.4us and
    the clock gate stays at 8/8.
  - softmax normalization is per-head-pair: denominators staged to
    partitions {0,64} (DVE partition-base alignment), one reciprocal per
    pair, selector matmul broadcasts 1/den; runs under the next pair's
    exp wall instead of 24 flat-cost 3.3us reciprocals on the tail.
  - output projection contracts head PAIRS (K=128) from the pair-stacked
    attention-output layout.
"""

import numpy as np

import concourse.bass as bass
import concourse.mybir as mybir
import concourse.tile as tile
from concourse.bass_utils import run_bass_kernel_spmd
from concourse.masks import make_identity

F32 = mybir.dt.float32
BF16 = mybir.dt.bfloat16
AF = mybir.ActivationFunctionType

B = 2
N = 4096          # tokens per batch
D = 768           # model dim
H = 12            # heads
DH = 64           # head dim
INNER = H * DH    # 768
EPS = 1e-8
NQ = 1024         # query tokens per core
NCORES = 8
BLK = 512         # projection token block
KT = N // 128     # 32 key tiles of 128


def _split_multi_waits(nc):
    """This container's walrus accepts only ONE sync-wait per instruction.
    Hoist extra waits into standalone EVSEM instructions placed just before."""
    n = 0
    for f in nc.m.functions:
        for bb in f.blocks:
            insts = list(bb.instructions)
            out = []
            for inst in insts:
                si = inst.sync_info
                if si is not None and si.on_wait is not None and len(si.on_wait) > 1:
                    waits = list(si.on_wait)
                    for j, w in enumerate(waits[:-1]):
                        ev = mybir.InstEventSemaphore(
                            name=f"{inst.name}-evw{j}",
                            engine=inst.engine,
                            sync_info=mybir.SyncInfo(on_wait=[w], on_update=[]),
                        )
                        out.append(ev)
                        n += 1
                    si.on_wait = [waits[-1]]
                out.append(inst)
            bb.instructions = out
    return n


def _build_program(inv_scale):
    """Build the single SPMD Bass program. inv_scale: list of 12 floats."""
    nc = bass.Bass()
    xb = nc.declare_dram_parameter("xb", [N, D], F32, isOutput=False)
    qx = nc.declare_dram_parameter("qx", [NQ, D], F32, isOutput=False)
    wqkvT = nc.declare_dram_parameter("wqkvT", [D, 3 * INNER], F32, isOutput=False)
    woT = nc.declare_dram_parameter("woT", [INNER, D], F32, isOutput=False)
    bout = nc.declare_dram_parameter("bout", [1, D], F32, isOutput=False)
    selin = nc.declare_dram_parameter("selin", [128, 128], F32, isOutput=False)
    selqin = nc.declare_dram_parameter("selqin", [128, 128], F32, isOutput=False)
    y = nc.declare_dram_parameter("y", [NQ, D], F32, isOutput=True)

    with tile.TileContext(nc) as tc:
        with tc.tile_pool(name="const", bufs=1) as constp, \
             tc.tile_pool(name="persist", bufs=1) as persist:
            # --- constants ---
            ident = constp.tile([128, 128], F32)
            make_identity(nc, ident)
            sel_bf = constp.tile([128, 128], BF16)
            selq_bf = constp.tile([128, 128], BF16)
            ones_f = constp.tile([1, 64], F32)
            nc.vector.memset(ones_f, 1.0)
            ones_bf = constp.tile([1, 128], BF16)
            nc.vector.memset(ones_bf, 1.0)
            eps_t = constp.tile([128, 1], F32)
            nc.vector.memset(eps_t, EPS)
            invs = constp.tile([128, 6], F32)
            for dt in range(6):
                nc.vector.memset(invs[0:64, dt:dt + 1], float(inv_scale[2 * dt]))
                nc.vector.memset(invs[64:128, dt:dt + 1],
                                 float(inv_scale[2 * dt + 1]))
            b_bf = constp.tile([1, D], BF16)

            # --- persistent activations ---
            khat = persist.tile([128, 6, N], BF16)     # k^T normed [dim, tok]
            qhat = persist.tile([128, H, NQ], BF16)    # q^T per head, zero-pad
            vhat = persist.tile([128, KT, H * 65], BF16)  # v [tok, h*65] (+ones)
            o2_all = persist.tile([128, 6, NQ], BF16)  # attn out, pair-stacked
            wo12 = persist.tile([128, 6, D], BF16)     # w_out pair-stacked

            # ---------------- phase W+P: weights, projections, head-norm ----
            with tc.tile_pool(name="pw", bufs=1) as pwp:
              wq = pwp.tile([128, 6, 3 * INNER], BF16)
              with tc.tile_pool(name="wstage", bufs=2) as wst, \
                   tc.tile_pool(name="cstage", bufs=1) as cst:
                # qkv-weight DMAs go first on the DVE-triggered queue so the
                # x-block loads (sync queue) run in parallel; casts on the
                # otherwise-idle ScalarE keep the DVE head clear for the
                # first transpose drains.
                for dt in range(6):
                    for third in range(3):
                        cs = bass.ts(3 * dt + third, INNER)
                        st = wst.tile([128, INNER], F32, tag="wst",
                                      name=f"wst_{dt}_{third}")
                        eng = nc.gpsimd if (dt % 2 == 0) else nc.scalar
                        eng.dma_start(
                            out=st, in_=wqkvT[dt * 128:(dt + 1) * 128,
                                              third * INNER:(third + 1) * INNER])
                        nc.scalar.copy(
                            wq[:, dt, third * INNER:(third + 1) * INNER], st)
                sel_st = cst.tile([128, 128], F32, tag="sel")
                nc.sync.dma_start(out=sel_st, in_=selin[:, :])
                nc.vector.tensor_copy(sel_bf, sel_st)
              with tc.tile_pool(name="wostage", bufs=1) as wost, \
                   tc.tile_pool(name="psmall", bufs=1) as psmall, \
                   tc.tile_pool(name="pksq", bufs=1) as pksq, \
                   tc.tile_pool(name="pkf", bufs=1) as pkf, \
                   tc.tile_pool(name="pstage", bufs=2) as pstage, \
                   tc.tile_pool(name="pxT", bufs=2) as pxT, \
                   tc.tile_pool(name="psumTP", bufs=1, space="PSUM") as pTP, \
                   tc.tile_pool(name="psumKP", bufs=2, space="PSUM") as pKP, \
                   tc.tile_pool(name="psumSQ", bufs=1, space="PSUM") as pSQ, \
                   tc.tile_pool(name="psumVP", bufs=2, space="PSUM") as pVP:

                  def load_transpose(src, blk_i):
                      # DMA x block [512, D] + transpose -> xT [dim, tok] bf16.
                      # Emitted one block ahead of the compute part so the
                      # PE's projection matmuls never wait on the DVE drains.
                      xsts = []
                      for hh in range(2):
                          xst = pstage.tile([128, 2, D], F32, tag="xst")
                          nc.sync.dma_start(
                              out=xst,
                              in_=src[blk_i * BLK + hh * 256:
                                      blk_i * BLK + (hh + 1) * 256, :].rearrange(
                                  "(t p) d -> p t d", p=128),
                          )
                          xsts.append(xst)
                      xT = pxT.tile([128, 6, BLK], BF16, tag="xT")
                      for dt in range(6):
                          tp = pTP.tile([128, 512], F32, tag="pTP")
                          for tt in range(4):
                              nc.tensor.transpose(
                                  tp[:, tt * 128:(tt + 1) * 128],
                                  xsts[tt // 2][:, tt % 2,
                                                dt * 128:(dt + 1) * 128], ident)
                          nc.vector.tensor_copy(xT[:, dt, :], tp)
                      return xT

                  def proj_compute(xT, blk_i, is_q):
                      wbase = 0 if is_q else INNER
                      # q^T / k^T projection [dim_out, tok]; kf bf16 via
                      # ScalarE, squares straight from PSUM via ScalarE Square
                      kf = pkf.tile([128, 6, BLK], BF16, tag="kf")
                      sq = pSQ.tile([128, 512], F32, tag="pSQ")
                      for dt in range(6):
                          kp = pKP.tile([128, 512], F32, tag="pKP")
                          for ks in range(6):
                              nc.tensor.matmul(
                                  kp,
                                  wq[:, ks, wbase + dt * 128: wbase + (dt + 1) * 128],
                                  xT[:, ks, :],
                                  start=(ks == 0), stop=(ks == 5))
                          nc.scalar.copy(kf[:, dt, :], kp)
                          ksq = pksq.tile([128, BLK], BF16, tag="ksq")
                          nc.scalar.square(ksq, kp)
                          nc.tensor.matmul(sq, sel_bf, ksq,
                                           start=(dt == 0), stop=(dt == 5))
                      nrm = psmall.tile([128, BLK], F32, tag="nrm")
                      nc.scalar.activation(nrm, sq, AF.Sqrt)
                      u = pSQ.tile([128, BLK], F32, tag="pSQ")
                      nc.scalar.activation(u, nrm, AF.Sqrt, bias=eps_t[:, :])
                      rq = psmall.tile([128, BLK], F32, tag="nrm")
                      nc.vector.reciprocal(rq, u)
                      bsl = bass.ts(blk_i, BLK)
                      if is_q:
                          # zero-padded per-head layout: head 2dt on rows 0:64,
                          # head 2dt+1 on rows 64:128, other rows stay zero.
                          # 1/scale_h is folded in so exp needs no scale;
                          # (kf * invs) * rq fused in one pass per half.
                          from concourse.alu_op_type import AluOpType as ALU
                          for dt in range(6):
                              nc.vector.scalar_tensor_tensor(
                                  qhat[0:64, 2 * dt, bsl],
                                  kf[0:64, dt, :], invs[0:64, dt:dt + 1],
                                  rq[0:64, :], ALU.mult, ALU.mult)
                              nc.vector.scalar_tensor_tensor(
                                  qhat[64:128, 2 * dt + 1, bsl],
                                  kf[64:128, dt, :], invs[64:128, dt:dt + 1],
                                  rq[64:128, :], ALU.mult, ALU.mult)
                          return
                      for dt in range(6):
                          nc.vector.tensor_mul(
                              khat[:, dt, bsl], kf[:, dt, :], rq)
                      # v projection [tok, inner] -> vhat strided 65
                      for tt in range(4):
                          vp = pVP.tile([128, 1024], F32, tag="pVP")
                          for ks in range(6):
                              nc.tensor.matmul(vp[:, 0:512],
                                               xT[:, ks, tt * 128:(tt + 1) * 128],
                                               wq[:, ks, 2 * INNER:2 * INNER + 512],
                                               start=(ks == 0), stop=(ks == 5))
                              nc.tensor.matmul(vp[:, 512:768],
                                               xT[:, ks, tt * 128:(tt + 1) * 128],
                                               wq[:, ks, 2 * INNER + 512:3 * INNER],
                                               start=(ks == 0), stop=(ks == 5))
                          vdst = vhat[:, blk_i * 4 + tt, :].rearrange(
                              "p (h c) -> p h c", c=65)[:, :, 0:64]
                          nc.vector.tensor_copy(
                              vdst, vp[:, 0:768].rearrange("p (h c) -> p h c", c=64))

                  # software pipeline: transpose part of block i+1 lands in
                  # the engine queues before the norm chain of block i
                  def stage_wo(h):
                      wst_t = wost.tile([64, D], F32, tag="wost")
                      nc.scalar.dma_start(out=wst_t,
                                          in_=woT[h * 64:(h + 1) * 64, :])
                      base = (h % 2) * 64
                      nc.vector.tensor_copy(wo12[base:base + 64, h // 2, :],
                                            wst_t)

                  staging = [lambda h=h: stage_wo(h) for h in range(H)]

                  blocks = [(qx, b, True) for b in range(NQ // BLK)] + \
                           [(xb, b, False) for b in range(N // BLK)]
                  xT_pend = load_transpose(blocks[0][0], blocks[0][1])
                  vones = vhat.rearrange(
                      "p t (h c) -> p t h c", c=65)[:, :, :, 64:65]
                  nc.vector.memset(vones, 1.0)
                  # qhat zero-padding rows, spread across the k-blocks so the
                  # DVE never stalls the next block's transpose drains
                  zeroing = []
                  for dt in range(6):
                      zeroing.append(qhat[64:128, 2 * dt, :])
                      zeroing.append(qhat[0:64, 2 * dt + 1, :])
                  for i, (src, b, is_q) in enumerate(blocks):
                      xT_cur = xT_pend
                      if i + 1 < len(blocks):
                          nsrc, nb, _ = blocks[i + 1]
                          xT_pend = load_transpose(nsrc, nb)
                      proj_compute(xT_cur, b, is_q)
                      if not is_q:
                          for _ in range(2):
                              if zeroing:
                                  nc.vector.memset(zeroing.pop(0), 0.0)
                      if i >= 2:
                          for _ in range(2):
                              if staging:
                                  staging.pop(0)()

            # ---------------- phase A: attention ----------------
            with tc.tile_pool(name="pP", bufs=6) as pP, \
                 tc.tile_pool(name="pden", bufs=2) as pden, \
                 tc.tile_pool(name="princ", bufs=2) as princ, \
                 tc.tile_pool(name="psumS", bufs=2, space="PSUM") as pS, \
                 tc.tile_pool(name="psumO", bufs=4, space="PSUM") as pO:
                # head-pair processing: heads (2hp, 2hp+1) live on PE row
                # groups 0-63 / 64-127 via the zero-padded qhat. Queries in
                # 512-halves so every PSUM tile is one bank.
                for nm, dst, srcdram, shp in (
                        ("selq", selq_bf, selqin, [128, 128]),
                        ("bias", b_bf, bout, [1, D])):
                    stg = pden.tile(shp, F32, tag="den", name=f"stg_{nm}")
                    nc.scalar.dma_start(out=stg, in_=srcdram[:, :])
                    nc.vector.tensor_copy(dst, stg)

                def norm_pair(hp, ots):
                    # drain + normalize pair hp: denominators staged to
                    # partitions {0, 64}, one reciprocal, selector matmul
                    # broadcasts 1/den. Deferred into the NEXT pair's kb
                    # stream so the PE queue never idles on the DVE chain.
                    den_st = pden.tile([128, 1024], F32, tag="den",
                                       name=f"den_{hp}")
                    nc.vector.memset(den_st, 1.0)
                    for j in range(2):
                        for qh in range(2):
                            qsl = bass.ts(qh, 512)
                            nc.vector.tensor_copy(
                                o2_all[64 * j:64 * j + 64, hp, qsl],
                                ots[(j, qh)][0:64, :])
                            nc.vector.tensor_copy(
                                den_st[64 * j:64 * j + 1, qsl],
                                ots[(j, qh)][64:65, :])
                    rinv = princ.tile([128, 1024], BF16, tag="rinv",
                                      name=f"rinv_{hp}")
                    with nc.allow_low_precision(
                            reason="bf16 1/den: 0.4% rel on softmax scale, "
                                   "well under the 2e-2 budget"):
                        nc.vector.reciprocal(rinv, den_st)
                    for qh in range(2):
                        qsl = bass.ts(qh, 512)
                        rbc = pS.tile([128, 512], F32, tag="pS",
                                      name=f"rbc_{qh}")
                        nc.tensor.matmul(rbc, selq_bf, rinv[:, qsl],
                                         start=True, stop=True)
                        nc.vector.tensor_mul(
                            o2_all[:, hp, qsl], o2_all[:, hp, qsl], rbc)

                pending = None
                for hp in range(6):
                    ots = {}
                    for j in range(2):
                        for qh in range(2):
                            ots[(j, qh)] = pO.tile([65, 512], F32, tag="pO",
                                                   name=f"ot_{hp}_{j}_{qh}")
                    for kb in range(KT):
                        kbsl = bass.ts(kb, 128)
                        sts = {}
                        for qh in range(2):
                            qsl = bass.ts(qh, 512)
                            st = pS.tile([128, 1024], F32, tag="pS",
                                         name=f"st_{qh}")
                            for j in range(2):
                                nc.tensor.matmul(
                                    st[:, j * 512:(j + 1) * 512],
                                    khat[:, hp, kbsl],
                                    qhat[:, 2 * hp + j, qsl],
                                    start=True, stop=True)
                            sts[qh] = st
                        for qh in range(2):
                            pt = pP.tile([128, 1024], BF16, tag="pP",
                                         name=f"pt_{qh}")
                            nc.scalar.activation(pt, sts[qh], AF.Exp)
                            for j in range(2):
                                h = 2 * hp + j
                                nc.tensor.matmul(
                                    ots[(j, qh)],
                                    vhat[:, kb, h * 65:(h + 1) * 65],
                                    pt[:, j * 512:(j + 1) * 512],
                                    start=(kb == 0), stop=(kb == KT - 1))
                        if kb == 6 and pending is not None:
                            pending()
                            pending = None
                    pending = (lambda hp=hp, ots=ots: norm_pair(hp, ots))
                pending()

            # ---------------- phase Y: output projection ----------------
            with tc.tile_pool(name="pys", bufs=2) as pys, \
                 tc.tile_pool(name="psumY", bufs=2, space="PSUM") as pY:
                for mt in range(NQ // 128):
                    mtsl = bass.ts(mt, 128)
                    yp = pY.tile([128, 1024], F32, tag="pY")
                    for hp in range(6):
                        lhsT = o2_all[:, hp, mtsl]
                        nc.tensor.matmul(yp[:, 0:512], lhsT, wo12[:, hp, 0:512],
                                         start=(hp == 0), stop=False)
                        nc.tensor.matmul(yp[:, 512:768], lhsT, wo12[:, hp, 512:768],
                                         start=(hp == 0), stop=False)
                    nc.tensor.matmul(yp[:, 0:512], ones_bf, b_bf[:, 0:512],
                                     start=False, stop=True)
                    nc.tensor.matmul(yp[:, 512:768], ones_bf, b_bf[:, 512:768],
                                     start=False, stop=True)
                    ys = pys.tile([128, D], F32, tag="ys")
                    nc.vector.tensor_copy(ys, yp[:, 0:768])
                    nc.sync.dma_start(out=y[mt * 128:(mt + 1) * 128, :], in_=ys)

    _split_multi_waits(nc)
    return nc


_prog_cache = {}


def _make_in_maps(inputs):
    x = np.ascontiguousarray(np.asarray(inputs["x"], dtype=np.float32))
    w_qkv = np.asarray(inputs["w_qkv"], dtype=np.float32)
    w_out = np.asarray(inputs["w_out"], dtype=np.float32)
    b_out = np.asarray(inputs["b_out"], dtype=np.float32).reshape(1, D)

    wqkvT = np.ascontiguousarray(w_qkv.T)            # [768, 2304]
    woT = np.ascontiguousarray(w_out.T)              # [768, 768]
    p = np.arange(128)
    sel = (p[:, None] % 64 == p[None, :] % 64).astype(np.float32)
    # selq[r, c] = 1 where r == 64*(c >= 64): broadcasts the 1/den rows
    # (staged at partitions 0/64) to the pair-stacked [128, q] layout
    selq = (p[:, None] == 64 * (p[None, :] >= 64)).astype(np.float32)

    in_maps = []
    for cix in range(NCORES):
        bi, qi = cix // 4, cix % 4
        in_maps.append({
            "xb": x[bi],
            "qx": np.ascontiguousarray(x[bi, qi * NQ:(qi + 1) * NQ]),
            "wqkvT": wqkvT,
            "woT": woT,
            "bout": b_out,
            "selin": sel,
            "selqin": selq,
        })
    return in_maps


def kernel(x, w_qkv, w_out, b_out, scale):
    scale = np.asarray(scale, dtype=np.float32)
    inv_scale = tuple(float(1.0 / s) for s in scale)
    nc = _prog_cache.get(inv_scale)
    if nc is None:
        nc = _build_program(inv_scale)
        _prog_cache[inv_scale] = nc

    in_maps = _make_in_maps(
        {"x": x, "w_qkv": w_qkv, "w_out": w_out, "b_out": b_out})

    res = run_bass_kernel_spmd(nc, in_maps, core_ids=list(range(NCORES)))
    out = np.empty((B, N, D), dtype=np.float32)
    for c in range(NCORES):
        bi, qi = c // 4, c % 4
        out[bi, qi * NQ:(qi + 1) * NQ] = res.results[c]["y"]
    return out
